# revision 12
# baseline (speedup 1.0000x reference)
"""Trainium2 Bass kernel for nn_Critic (bidirectional-LSTM critic network).

Data-parallel over the B (sequence) dimension: 8 NeuronCores x 512 sequences.
Per core, feature-major layout throughout:

  Phase T (trunk): two LayerNorm-MLP layers. Mean-centering is folded into
    the weights on the host (W @ (I - 1/64)), so LN reduces to an rsqrt of
    the per-sample sum-of-squares, computed with PE reduce/replicate matmuls.
    Timestep blocks are emitted two-ended (t, 63-t) so both LSTM directions
    could stream (phases are serialized in v1 due to ACT table sets).
  Phase L (LSTM): 64 steps, fw+bw packed on partitions [128 = 64fw|64bw, 512].
    Gates come from col-tiled matmuls; sigmoid/tanh on ACT; cell update on
    DVE; per-step head dot-products (wx, wp) via tiny matmuls, staged to DRAM.
  Phase H (head): per-row LayerNorm over T, softmax(pn @ W3) attention,
    weighted sum -> [2B] output.
"""

import os
import sys

for _p in ("/opt/trn_rl_repo",):
    if _p not in sys.path:
        sys.path.insert(0, _p)

import json as _json
from types import MethodType as _MethodType

import numpy as np

import concourse.bass as bass
import concourse.tile as tile
from concourse import mybir
from concourse.bass_utils import run_bass_kernel_spmd

F32 = mybir.dt.float32
AF = mybir.ActivationFunctionType
ALU = mybir.AluOpType

B, T, H, OBS, ACTD = 4096, 64, 64, 128, 32
NCORES = 8
BC = B // NCORES  # 512 sequences per core
NT = T * BC  # 32768 rows per core
EPS = 1e-12

# ---------------------------------------------------------------- waitfix --
# This walrus build rejects instructions carrying more than one sync-wait
# command. The Tile kernel-tail drain (and barriers) routinely carry more.
# Patch the serialized BIR: move excess waits onto inserted NoOp carriers.
_MAX_WAITS = 1


def _patch_bir(bir):
    n = [0]

    def fresh():
        n[0] += 1
        return f"I-waitfix-{n[0]}"

    for fn in bir.get("functions", []):
        for bb in fn.get("blocks", []):
            out = []
            for inst in bb.get("instructions", []):
                si = inst.get("sync_info") or {}
                waits = si.get("on_wait") or []
                if len(waits) > _MAX_WAITS:
                    extra = waits[: len(waits) - _MAX_WAITS]
                    keep = waits[len(waits) - _MAX_WAITS :]
                    for i in range(0, len(extra), _MAX_WAITS):
                        out.append(
                            {
                                "name": fresh(),
                                "opcode": "NoOp",
                                "engine": inst["engine"],
                                "ins": [],
                                "outs": [],
                                "sync_info": {
                                    "on_wait": extra[i : i + _MAX_WAITS],
                                    "on_update": [],
                                },
                            }
                        )
                    si = dict(si)
                    si["on_wait"] = keep
                    inst = dict(inst)
                    inst["sync_info"] = si
                out.append(inst)
            bb["instructions"] = out
    return bir


def _install_waitfix(nc):
    orig = nc.to_json_bytes

    def patched(self):
        return _json.dumps(_patch_bir(_json.loads(orig()))).encode()

    nc.to_json_bytes = _MethodType(patched, nc)
    return nc


# ----------------------------------------------------------- host weights --


def _prep_consts(p):
    """Fold reference weights into device layouts. p: dict of np arrays."""
    f32 = lambda x: np.ascontiguousarray(x, dtype=np.float32)
    C = np.eye(64, dtype=np.float64) - 1.0 / 64.0

    c = {}
    c["W1c"] = f32(p["W1"].astype(np.float64) @ C)  # [128, 64]
    c["b1c"] = f32(p["b1"].astype(np.float64) @ C)  # [64]
    c["W2c"] = f32(p["W2"].astype(np.float64) @ C)  # [96, 64]
    c["b2c"] = f32(p["b2"].astype(np.float64) @ C)  # [64]
    c["g1be1"] = f32(np.stack([np.tile(p["g1"], 2), np.tile(p["be1"], 2)], 1))
    c["g2be2"] = f32(np.stack([np.tile(p["g2"], 2), np.tile(p["be2"], 2)], 1))

    # Gate weights: lhsT rows = xh rows = [h (64); x (64)]; reference W rows =
    # [x (64); h (64)].  Columns i|j|f|o stay in place.
    c["Wgf"] = f32(np.concatenate([p["Wf"][64:128], p["Wf"][0:64]], 0))
    c["Wgb"] = f32(np.concatenate([p["Wb"][64:128], p["Wb"][0:64]], 0))
    gb = np.zeros((128, 4), np.float64)
    for gi, sl in enumerate((slice(0, 64), slice(64, 128), slice(128, 192), slice(192, 256))):
        gb[0:64, gi] = p["bf"][sl]
        gb[64:128, gi] = p["bb"][sl]
    gb[:, 2] += 1.0  # forget_bias
    c["gbias"] = f32(gb)  # cols: i, j, f, o

    c["whead"] = f32(np.stack([p["wx"], p["wp"]], 1))  # [64, 2]
    c["CT"] = f32(np.eye(64) - 1.0 / 64.0)
    c["ones_red"] = f32(
        np.block(
            [[np.ones((64, 1)), np.zeros((64, 1))], [np.zeros((64, 1)), np.ones((64, 1))]]
        )
    )  # [128, 2]
    c["ones_rep"] = f32(
        np.block(
            [[np.ones((1, 64)), np.zeros((1, 64))], [np.zeros((1, 64)), np.ones((1, 64))]]
        )
    )  # [2, 128]
    c["ones64"] = f32(np.ones((64, 1)))
    c["ones_1_64"] = f32(np.ones((1, 64)))

    # Head params; bw direction is time-reversed relative to our bw scan
    # state order, handled by reversing the per-t parameters.
    c["W3f"] = f32(p["W3"])
    c["W3b"] = f32(p["W3"][::-1, ::-1])
    c["b3f"] = f32(p["b3"].reshape(64, 1))
    c["b3b"] = f32(p["b3"][::-1].reshape(64, 1))
    c["gpbepf"] = f32(np.stack([p["gp"], p["bep"]], 1))  # [64, 2]
    c["gpbepb"] = f32(np.stack([p["gp"][::-1], p["bep"][::-1]], 1))
    c["bx"] = float(np.asarray(p["bx"]))
    return c


# ------------------------------------------------------------ bass program --


def _build(consts):
    nc = bass.Bass()
    obsT = nc.declare_dram_parameter("obsT", [128, NT], F32, isOutput=False)
    actT = nc.declare_dram_parameter("actT", [32, NT], F32, isOutput=False)

    cin = {}
    for name in (
        "W1c", "b1c", "W2c", "b2c", "g1be1", "g2be2", "Wgf", "Wgb", "gbias",
        "whead", "CT", "ones_red", "ones_rep", "ones64", "ones_1_64",
        "W3f", "W3b", "b3f", "b3b", "gpbepf", "gpbepb",
    ):
        a = consts[name]
        shp = list(a.shape) if a.ndim == 2 else [a.shape[0], 1]
        cin[name] = nc.declare_dram_parameter(name, shp, F32, isOutput=False)

    ov = nc.declare_dram_parameter("ov", [2, BC], F32, isOutput=True)
    xs_dram = nc.dram_tensor("xs_stash", [4, NT], F32)

    b1_nz = bool(np.any(consts["b1c"]))
    b2_nz = bool(np.any(consts["b2c"]))

    with tile.TileContext(nc) as tc:
        with (
            tc.tile_pool(name="singles", bufs=1) as sing,
            tc.tile_pool(name="obs_p", bufs=4) as obs_p,
            tc.tile_pool(name="act_p", bufs=4) as act_p,
            tc.tile_pool(name="big", bufs=2) as big,
            tc.tile_pool(name="small", bufs=4) as small,
        ):
            # ---- load constants ----
            ct = {}
            for name, dram in cin.items():
                a = consts[name]
                shp = list(a.shape) if a.ndim == 2 else [a.shape[0], 1]
                ct[name] = sing.tile(shp, F32, name=f"ct_{name}", tag=f"ct_{name}")
                nc.sync.dma_start(out=ct[name], in_=dram[:, :])

            X2 = sing.tile([128, 32 * BC], F32)  # x2, two-ended t-pair packing
            epst = sing.tile([128, 1], F32)
            nc.vector.memset(epst, EPS)

            # b1c/b2c as [128,1] per-partition (2-stacked) if needed
            if b1_nz or b2_nz:
                bstk = sing.tile([128, 2], F32)
                # col 0 = [b1c;b1c], col 1 = [b2c;b2c] via DMA broadcast: the
                # host passes b1c/b2c as [64,1]; replicate by two DMAs each.
                for col, nm in ((0, "b1c"), (1, "b2c")):
                    nc.sync.dma_start(out=bstk[0:64, col : col + 1], in_=cin[nm][:, :])
                    nc.sync.dma_start(out=bstk[64:128, col : col + 1], in_=cin[nm][:, :])

            # ================= Phase T: trunk =================
            scope_T, _ = nc.enter_named_scope("phaseT", False)
            trunk_ps = tc.tile_pool(name="trunk_ps", bufs=2, space="PSUM")
            with trunk_ps as ps_v, tc.tile_pool(name="trunk_ps2", bufs=2, space="PSUM") as ps_s, tc.tile_pool(name="trunk_ps3", bufs=2, space="PSUM") as ps_r:
              for j in range(32):
                  ta, tb = j, 63 - j
                  oa = obs_p.tile([128, BC], F32, tag="obs")
                  nc.sync.dma_start(out=oa, in_=obsT[:, ta * BC : (ta + 1) * BC])
                  ob = obs_p.tile([128, BC], F32, tag="obs")
                  nc.sync.dma_start(out=ob, in_=obsT[:, tb * BC : (tb + 1) * BC])

                  # ---- layer 1 ----
                  pv = ps_v.tile([128, BC], F32, tag="pv")
                  nc.tensor.matmul(pv[0:64, :], ct["W1c"], oa, start=True, stop=True,
                                   tile_position=(0, 0))
                  nc.tensor.matmul(pv[64:128, :], ct["W1c"], ob, start=True, stop=True,
                                   tile_position=(0, 64))
                  vsq = big.tile([128, BC], F32, tag="vsq")
                  v_s = big.tile([128, BC], F32, tag="v_s")
                  if b1_nz:
                      nc.scalar.activation(vsq, pv, AF.Square, bias=bstk[:, 0:1])
                      nc.scalar.activation(v_s, pv, AF.Identity, bias=bstk[:, 0:1])
                  else:
                      nc.scalar.activation(vsq, pv, AF.Square)
                      nc.vector.tensor_copy(v_s, pv)
                  pss = ps_s.tile([2, BC], F32, tag="pss")
                  nc.tensor.matmul(pss, ct["ones_red"], vsq, start=True, stop=True)
                  stdv = small.tile([2, BC], F32, tag="stdv")
                  nc.scalar.activation(stdv, pss, AF.Sqrt, bias=epst[0:2, 0:1], scale=1.0 / 64.0)
                  rstd = small.tile([2, BC], F32, tag="rstd")
                  nc.vector.reciprocal(rstd, stdv)
                  prep = ps_r.tile([128, BC], F32, tag="prep")
                  nc.tensor.matmul(prep, ct["ones_rep"], rstd, start=True, stop=True)
                  xn = big.tile([128, BC], F32, tag="xn")
                  nc.vector.tensor_mul(xn, v_s, prep)
                  xa = big.tile([96, BC], F32, tag="xa")
                  xb = big.tile([96, BC], F32, tag="xb")
                  nc.scalar.activation(xa[0:64, :], xn[0:64, :], AF.Relu,
                                       bias=ct["g1be1"][0:64, 1:2],
                                       scale=ct["g1be1"][0:64, 0:1])
                  nc.scalar.activation(xb[0:64, :], xn[64:128, :], AF.Relu,
                                       bias=ct["g1be1"][64:128, 1:2],
                                       scale=ct["g1be1"][64:128, 0:1])
                  nc.sync.dma_start(out=xa[64:96, :], in_=actT[:, ta * BC : (ta + 1) * BC])
                  nc.sync.dma_start(out=xb[64:96, :], in_=actT[:, tb * BC : (tb + 1) * BC])

                  # ---- layer 2 ----
                  pv2 = ps_v.tile([128, BC], F32, tag="pv")
                  nc.tensor.matmul(pv2[0:64, :], ct["W2c"], xa[0:96, :], start=True,
                                   stop=True, tile_position=(0, 0))
                  nc.tensor.matmul(pv2[64:128, :], ct["W2c"], xb[0:96, :], start=True,
                                   stop=True, tile_position=(0, 64))
                  vsq2 = big.tile([128, BC], F32, tag="vsq")
                  v_s2 = big.tile([128, BC], F32, tag="v_s")
                  if b2_nz:
                      nc.scalar.activation(vsq2, pv2, AF.Square, bias=bstk[:, 1:2])
                      nc.scalar.activation(v_s2, pv2, AF.Identity, bias=bstk[:, 1:2])
                  else:
                      nc.scalar.activation(vsq2, pv2, AF.Square)
                      nc.vector.tensor_copy(v_s2, pv2)
                  pss2 = ps_s.tile([2, BC], F32, tag="pss")
                  nc.tensor.matmul(pss2, ct["ones_red"], vsq2, start=True, stop=True)
                  stdv2 = small.tile([2, BC], F32, tag="stdv")
                  nc.scalar.activation(stdv2, pss2, AF.Sqrt, bias=epst[0:2, 0:1], scale=1.0 / 64.0)
                  rstd2 = small.tile([2, BC], F32, tag="rstd")
                  nc.vector.reciprocal(rstd2, stdv2)
                  prep2 = ps_r.tile([128, BC], F32, tag="prep")
                  nc.tensor.matmul(prep2, ct["ones_rep"], rstd2, start=True, stop=True)
                  xn2 = big.tile([128, BC], F32, tag="xn")
                  nc.vector.tensor_mul(xn2, v_s2, prep2)
                  # write both t-blocks of x2 into X2 block j in one op
                  nc.scalar.activation(X2[:, j * BC : (j + 1) * BC], xn2, AF.Relu,
                                       bias=ct["g2be2"][:, 1:2],
                                       scale=ct["g2be2"][:, 0:1])

            nc.leave_named_scope("phaseT", scope_T, False)
            tc.strict_bb_all_engine_barrier()

            # ================= Phase L: LSTM =================
            scope_L, _ = nc.enter_named_scope("phaseL", False)
            xh_f = sing.tile([128, BC], F32)  # rows 0:64 h_fw, 64:128 x_fw
            xh_b = sing.tile([128, BC], F32)
            cst = sing.tile([128, BC], F32)  # cell state [c_fw; c_bw]
            nc.vector.memset(xh_f[0:64, :], 0.0)
            nc.vector.memset(xh_b[0:64, :], 0.0)
            nc.vector.memset(cst, 0.0)

            lstm_ps_cm = tc.tile_pool(name="lstm_ps", bufs=1, space="PSUM")
            lstm_ps = lstm_ps_cm.__enter__()
            pg_i = lstm_ps.tile([128, BC], F32, tag="pg_i")
            pg_j = lstm_ps.tile([128, BC], F32, tag="pg_j")
            pg_f = lstm_ps.tile([128, BC], F32, tag="pg_f")
            pg_o = lstm_ps.tile([128, BC], F32, tag="pg_o")
            ph_f = [lstm_ps.tile([2, 2 * BC], F32, name=f"ph_f{i}", tag=f"ph_f{i}") for i in range(1)]
            ph_b = [lstm_ps.tile([2, 2 * BC], F32, name=f"ph_b{i}", tag=f"ph_b{i}") for i in range(1)]
            stg_f = [sing.tile([2, 2 * BC], F32, name=f"stg_f{i}", tag=f"stg_f{i}") for i in range(2)]
            stg_b = [sing.tile([2, 2 * BC], F32, name=f"stg_b{i}", tag=f"stg_b{i}") for i in range(2)]

            def xsrc(t_needed):
                if t_needed < 32:
                    return X2[0:64, t_needed * BC : (t_needed + 1) * BC]
                jj = 63 - t_needed
                return X2[64:128, jj * BC : (jj + 1) * BC]

            gates = ((pg_i, 0, AF.Sigmoid), (pg_j, 1, AF.Tanh),
                     (pg_f, 2, AF.Sigmoid), (pg_o, 3, AF.Sigmoid))

            for t in range(T):
                nc.vector.tensor_copy(xh_f[64:128, :], xsrc(t))
                nc.vector.tensor_copy(xh_b[64:128, :], xsrc(63 - t))
                for pg, gi, _fn in gates:
                    wf = ct["Wgf"][:, gi * 64 : (gi + 1) * 64]
                    wb = ct["Wgb"][:, gi * 64 : (gi + 1) * 64]
                    nc.tensor.matmul(pg[0:64, :], wf, xh_f, start=True, stop=True,
                                     tile_position=(0, 0))
                    nc.tensor.matmul(pg[64:128, :], wb, xh_b, start=True, stop=True,
                                     tile_position=(0, 64))
                sI = big.tile([128, BC], F32, tag="sI")
                tJ = big.tile([128, BC], F32, tag="tJ")
                sF = big.tile([128, BC], F32, tag="sF")
                sO = big.tile([128, BC], F32, tag="sO")
                for (pg, gi, fn), dst in zip(gates, (sI, tJ, sF, sO)):
                    nc.scalar.activation(dst, pg, fn, bias=ct["gbias"][:, gi : gi + 1])
                u = big.tile([128, BC], F32, tag="u")
                nc.vector.tensor_mul(u, sI, tJ)
                cf = big.tile([128, BC], F32, tag="cf")
                nc.vector.tensor_mul(cf, cst, sF)
                nc.vector.tensor_add(cst, cf, u)
                tcl = big.tile([128, BC], F32, tag="tc")
                nc.scalar.activation(tcl, cst, AF.Tanh)
                nc.vector.tensor_mul(xh_f[0:64, :], tcl[0:64, :], sO[0:64, :])
                nc.vector.tensor_mul(xh_b[0:64, :], tcl[64:128, :], sO[64:128, :])

                # head dots: [xs; ps] rows for this step
                ph = ph_f[0]
                pb = ph_b[0]
                col = (t % 2) * BC
                nc.tensor.matmul(ph[0:2, col : col + BC], ct["whead"], xh_f[0:64, :],
                                 start=True, stop=True, tile_position=(0, 0))
                nc.tensor.matmul(pb[0:2, col : col + BC], ct["whead"], xh_b[0:64, :],
                                 start=True, stop=True, tile_position=(0, 0))
                if t % 2 == 1:
                    sf = stg_f[(t // 2) % 2]
                    sb = stg_b[(t // 2) % 2]
                    nc.vector.tensor_copy(sf, ph)
                    nc.vector.tensor_copy(sb, pb)
                    dcol = (t - 1) * BC
                    nc.sync.dma_start(out=xs_dram[0:2, dcol : dcol + 2 * BC], in_=sf)
                    nc.sync.dma_start(out=xs_dram[2:4, dcol : dcol + 2 * BC], in_=sb)

            lstm_ps_cm.__exit__(None, None, None)
            nc.leave_named_scope("phaseL", scope_L, False)
            tc.strict_bb_all_engine_barrier()

            # ================= Phase H: head =================
            scope_H, _ = nc.enter_named_scope("phaseH", False)
            head_ps_cm = tc.tile_pool(name="head_ps", bufs=1, space="PSUM")
            ps_h = head_ps_cm.__enter__()
            pn_d = []
            xs_d = []
            for d, (w3, b3, gpb) in enumerate(
                (("W3f", "b3f", "gpbepf"), ("W3b", "b3b", "gpbepb"))
            ):
                xsT = big.tile([64, BC], F32, tag="u")
                psT = big.tile([64, BC], F32, tag="cf")
                nc.sync.dma_start(
                    out=xsT, in_=xs_dram[2 * d : 2 * d + 1, :].rearrange("o (t b) -> (o t) b", b=BC)
                )
                nc.sync.dma_start(
                    out=psT, in_=xs_dram[2 * d + 1 : 2 * d + 2, :].rearrange("o (t b) -> (o t) b", b=BC)
                )
                pc = ps_h.tile([64, BC], F32, tag="hpc")
                nc.tensor.matmul(pc, ct["CT"], psT, start=True, stop=True)
                hsq = big.tile([64, BC], F32, tag="vsq")
                hcs = big.tile([64, BC], F32, tag="v_s")
                nc.scalar.activation(hsq, pc, AF.Square)
                nc.vector.tensor_copy(hcs, pc)
                hss = ps_h.tile([1, BC], F32, tag="hss")
                nc.tensor.matmul(hss, ct["ones64"], hsq, start=True, stop=True)
                hstd = small.tile([1, BC], F32, tag="stdv")
                nc.scalar.activation(hstd, hss, AF.Sqrt, bias=epst[0:1, 0:1], scale=1.0 / 64.0)
                hrst = small.tile([1, BC], F32, tag="rstd")
                nc.vector.reciprocal(hrst, hstd)
                hrep = ps_h.tile([64, BC], F32, tag="hrep")
                nc.tensor.matmul(hrep, ct["ones_1_64"], hrst, start=True, stop=True)
                ht1 = big.tile([64, BC], F32, tag="xn")
                nc.vector.tensor_mul(ht1, hcs, hrep)
                pn = big.tile([64, BC], F32, tag="tc")
                nc.scalar.activation(pn, ht1, AF.Relu, bias=ct[gpb][:, 1:2],
                                     scale=ct[gpb][:, 0:1])
                pn_d.append((pn, w3, b3))
                xs_d.append(xsT)

            ovs0 = sing.tile([1, BC], F32)
            ovs1 = sing.tile([1, BC], F32)
            for d, ((pn, w3, b3), xsT) in enumerate(zip(pn_d, xs_d)):
                pl = ps_h.tile([64, BC], F32, tag="hpl")
                nc.tensor.matmul(pl, ct[w3], pn, start=True, stop=True)
                he = big.tile([64, BC], F32, tag="sI")
                nc.scalar.activation(he, pl, AF.Exp, bias=ct[b3][:, 0:1])
                hse = ps_h.tile([1, BC], F32, tag="hse")
                nc.tensor.matmul(hse, ct["ones64"], he, start=True, stop=True)
                hrs = small.tile([1, BC], F32, tag="rstd")
                nc.vector.reciprocal(hrs, hse)
                hex = big.tile([64, BC], F32, tag="tJ")
                nc.vector.tensor_mul(hex, he, xsT)
                hnum = ps_h.tile([1, BC], F32, tag="hnum")
                nc.tensor.matmul(hnum, ct["ones64"], hex, start=True, stop=True)
                hov = small.tile([1, BC], F32, tag="stdv")
                nc.vector.tensor_mul(hov, hrs, hnum)
                nc.vector.tensor_scalar(
                    ovs0 if d == 0 else ovs1, hov, float(consts["bx"]), None, ALU.add
                )
            nc.sync.dma_start(out=ov[0:1, :], in_=ovs0)
            nc.sync.dma_start(out=ov[1:2, :], in_=ovs1)
            head_ps_cm.__exit__(None, None, None)
            nc.leave_named_scope("phaseH", scope_H, False)

    return nc


_CACHE = {}


def kernel(**inputs):
    obs = np.ascontiguousarray(inputs["obs"], dtype=np.float32)
    action = np.ascontiguousarray(inputs["action"], dtype=np.float32)
    consts = _prep_consts(inputs)

    key = "nc"
    if key not in _CACHE:
        _CACHE[key] = _install_waitfix(_build(consts))
    nc = _CACHE[key]

    const_feed = {}
    for name, a in consts.items():
        if name == "bx":
            continue
        const_feed[name] = a if a.ndim == 2 else a.reshape(-1, 1)

    in_maps = []
    for c in range(NCORES):
        sl = slice(c * BC * T, (c + 1) * BC * T)
        obsT = np.ascontiguousarray(
            obs[sl].reshape(BC, T, OBS).transpose(2, 1, 0).reshape(OBS, NT)
        )
        actT = np.ascontiguousarray(
            action[sl].reshape(BC, T, ACTD).transpose(2, 1, 0).reshape(ACTD, NT)
        )
        m = {"obsT": obsT, "actT": actT}
        m.update(const_feed)
        in_maps.append(m)

    kw = {}
    if os.environ.get("BASS_TRACE"):
        kw["trace"] = True
        if os.environ.get("BASS_TRACE_DIR"):
            kw["tmpdir"] = os.environ["BASS_TRACE_DIR"]
    full = run_bass_kernel_spmd(nc, in_maps, list(range(NCORES)), **kw)
    globals()["LAST_RESULTS"] = full
    res = full.results

    out = np.empty(2 * B, dtype=np.float32)
    for c in range(NCORES):
        ovc = res[c]["ov"]
        out[c * BC : (c + 1) * BC] = ovc[0]
        out[B + c * BC : B + (c + 1) * BC] = ovc[1]
    return out



# revision 15
# speedup vs baseline: 1.5401x; 1.5401x over previous
"""Trainium2 Bass kernel for nn_Critic (bidirectional-LSTM critic network).

v2: all matmuls bf16 (1 cyc/row vs fp32's 4), LayerNorm via replicated
sum-of-squares matmul + Ln/Exp rsqrt (kills the 3.3us InstReciprocal),
relu+normalize fused into one DVE scalar_tensor_tensor, LSTM gates as a
single fused Sigmoid over a 4-bank PSUM tile (tanh(j) = 2*sigmoid(2j)-1
folded into weights), forget bias via K=1 accumulating matmul, cell-state
mul offloaded to GpSimd, head stash in bf16 with softmax reciprocal as
exp(-ln(x)).

Data-parallel over B: 8 NeuronCores x 512 sequences. Feature-major layout;
trunk processes timestep pairs (j, 63-j) packed on partitions [ta|tb].
"""

import os
import sys

for _p in ("/opt/trn_rl_repo",):
    if _p not in sys.path:
        sys.path.insert(0, _p)

import json as _json
from types import MethodType as _MethodType

import ml_dtypes
import numpy as np

import concourse.bass as bass
import concourse.tile as tile
from concourse import mybir
from concourse.bass_utils import run_bass_kernel_spmd

F32 = mybir.dt.float32
BF16 = mybir.dt.bfloat16
AF = mybir.ActivationFunctionType
ALU = mybir.AluOpType
BF = ml_dtypes.bfloat16

B, T, H, OBS, ACTD = 4096, 64, 64, 128, 32
NCORES = 8
BC = B // NCORES  # 512 sequences per core
NT = T * BC
EPS = 1e-12

# ---------------------------------------------------------------- waitfix --
_MAX_WAITS = 1


def _patch_bir(bir):
    n = [0]

    def fresh():
        n[0] += 1
        return f"I-waitfix-{n[0]}"

    for fn in bir.get("functions", []):
        for bb in fn.get("blocks", []):
            out = []
            for inst in bb.get("instructions", []):
                si = inst.get("sync_info") or {}
                waits = si.get("on_wait") or []
                if len(waits) > _MAX_WAITS:
                    extra = waits[: len(waits) - _MAX_WAITS]
                    keep = waits[len(waits) - _MAX_WAITS :]
                    for i in range(0, len(extra), _MAX_WAITS):
                        out.append(
                            {
                                "name": fresh(),
                                "opcode": "NoOp",
                                "engine": inst["engine"],
                                "ins": [],
                                "outs": [],
                                "sync_info": {
                                    "on_wait": extra[i : i + _MAX_WAITS],
                                    "on_update": [],
                                },
                            }
                        )
                    si = dict(si)
                    si["on_wait"] = keep
                    inst = dict(inst)
                    inst["sync_info"] = si
                out.append(inst)
            bb["instructions"] = out
    return bir


def _install_waitfix(nc):
    orig = nc.to_json_bytes

    def patched(self):
        return _json.dumps(_patch_bir(_json.loads(orig()))).encode()

    nc.to_json_bytes = _MethodType(patched, nc)
    return nc


# ----------------------------------------------------------- host weights --


def _prep_consts(p):
    f32 = lambda x: np.ascontiguousarray(x, dtype=np.float32)
    bf = lambda x: np.ascontiguousarray(np.asarray(x, np.float64), dtype=np.float32).astype(BF)
    C = np.eye(64, dtype=np.float64) - 1.0 / 64.0

    c = {}
    c["W1c"] = bf(np.asarray(p["W1"], np.float64) @ C)  # [128, 64]
    W2c = np.asarray(p["W2"], np.float64) @ C  # [96, 64]
    blk = np.zeros((128, 128), np.float64)
    blk[0:64, 0:64] = W2c[0:64]
    blk[64:128, 64:128] = W2c[0:64]
    c["W2X"] = bf(blk)
    blka = np.zeros((64, 128), np.float64)
    blka[0:32, 0:64] = W2c[64:96]
    blka[32:64, 64:128] = W2c[64:96]
    c["W2A"] = bf(blka)
    rep = np.zeros((128, 128), np.float64)
    rep[0:64, 0:64] = 1.0
    rep[64:128, 64:128] = 1.0
    c["REP"] = bf(rep)

    # trunk fast-path flags and general-path params
    c["b1_nz"] = bool(np.any(np.asarray(p["b1"])))
    c["b2_nz"] = bool(np.any(np.asarray(p["b2"])))
    b1c = np.asarray(p["b1"], np.float64) @ C
    b2c = np.asarray(p["b2"], np.float64) @ C
    c["bstk"] = f32(np.stack([np.tile(b1c, 2), np.tile(b2c, 2)], 1))  # [128,2]
    c["g1_triv"] = bool(
        np.all(np.asarray(p["g1"]) == 1) and np.all(np.asarray(p["be1"]) == 0)
    )
    c["g2_triv"] = bool(
        np.all(np.asarray(p["g2"]) == 1) and np.all(np.asarray(p["be2"]) == 0)
    )
    c["g1be1"] = f32(np.stack([np.tile(p["g1"], 2), np.tile(p["be1"], 2)], 1))
    c["g2be2"] = f32(np.stack([np.tile(p["g2"], 2), np.tile(p["be2"], 2)], 1))

    # LSTM gate weights: lhsT rows = [h (64); x (64)], j-gate scaled by 2
    # (tanh(z) = 2*sigmoid(2z) - 1).
    Wf = np.asarray(p["Wf"], np.float64)
    Wb = np.asarray(p["Wb"], np.float64)
    bfv = np.asarray(p["bf"], np.float64).copy()
    bbv = np.asarray(p["bb"], np.float64).copy()
    bfv[128:192] += 1.0  # forget_bias on f gate
    bbv[128:192] += 1.0
    gb = []
    for g in range(4):
        sl = slice(g * 64, (g + 1) * 64)
        scale = 2.0 if g == 1 else 1.0
        wgf = np.concatenate([Wf[64:128, sl], Wf[0:64, sl]], 0) * scale
        wgb = np.concatenate([Wb[64:128, sl], Wb[0:64, sl]], 0) * scale
        c[f"Wgf{g}"] = bf(wgf)
        c[f"Wgb{g}"] = bf(wgb)
        row = np.concatenate([bfv[sl], bbv[sl]]) * scale
        gb.append(row)
    c["gbias"] = np.stack(gb, 0)  # [4,128] f64, host-side only
    c["bias_nz"] = [bool(np.any(gb[g])) for g in range(4)]
    for g in range(4):
        c[f"bias{g}"] = bf(gb[g].reshape(1, 128))

    wx = np.asarray(p["wx"], np.float64)
    wp = np.asarray(p["wp"], np.float64)
    w4f = np.zeros((64, 4), np.float64)
    w4f[:, 0] = wx
    w4f[:, 1] = wp
    w4b = np.zeros((64, 4), np.float64)
    w4b[:, 2] = wx
    w4b[:, 3] = wp
    c["W4F"] = bf(w4f)
    c["W4B"] = bf(w4b)

    c["CT"] = bf(np.eye(64) - 1.0 / 64.0)
    c["REP64"] = bf(np.ones((64, 64)))
    c["ones64"] = bf(np.ones((64, 1)))
    c["W3f"] = bf(p["W3"])
    c["W3b"] = bf(np.asarray(p["W3"])[::-1, ::-1])
    c["b3f"] = f32(np.asarray(p["b3"]).reshape(64, 1))
    c["b3b"] = f32(np.asarray(p["b3"])[::-1].reshape(64, 1))
    c["gp_triv"] = bool(
        np.all(np.asarray(p["gp"]) == 1) and np.all(np.asarray(p["bep"]) == 0)
    )
    c["gpbepf"] = f32(np.stack([p["gp"], p["bep"]], 1))
    c["gpbepb"] = f32(np.stack([np.asarray(p["gp"])[::-1], np.asarray(p["bep"])[::-1]], 1))
    c["bx"] = float(np.asarray(p["bx"]))
    return c


# ------------------------------------------------------------ bass program --

_BF16_CONSTS = (
    ["W1c", "W2X", "W2A", "REP", "W4F", "W4B", "CT", "REP64", "ones64", "W3f", "W3b"]
    + [f"Wgf{g}" for g in range(4)]
    + [f"Wgb{g}" for g in range(4)]
    + [f"bias{g}" for g in range(4)]
)
_F32_CONSTS = ["bstk", "g1be1", "g2be2", "b3f", "b3b", "gpbepf", "gpbepb"]


def _build(consts):
    nc = bass.Bass()
    obsT = nc.declare_dram_parameter("obsT", [128, NT], BF16, isOutput=False)
    actT = nc.declare_dram_parameter("actT", [32, NT], BF16, isOutput=False)

    cin = {}
    for name in _BF16_CONSTS:
        cin[name] = nc.declare_dram_parameter(name, list(consts[name].shape), BF16, isOutput=False)
    for name in _F32_CONSTS:
        cin[name] = nc.declare_dram_parameter(name, list(consts[name].shape), F32, isOutput=False)

    ov = nc.declare_dram_parameter("ov", [2, BC], F32, isOutput=True)
    xs_dram = nc.dram_tensor("xs_stash", [4, NT], BF16)

    with tile.TileContext(nc) as tc:
        with (
            tc.tile_pool(name="singles", bufs=1) as sing,
            tc.tile_pool(name="io_p", bufs=3) as io_p,
            tc.tile_pool(name="big", bufs=2) as big,
        ):
            ct = {}
            for name in _BF16_CONSTS + _F32_CONSTS:
                ct[name] = sing.tile(
                    list(consts[name].shape),
                    BF16 if name in _BF16_CONSTS else F32,
                    name=f"ct_{name}",
                    tag=f"ct_{name}",
                )
                nc.sync.dma_start(out=ct[name], in_=cin[name][:, :])

            X2 = sing.tile([128, 32 * BC], BF16)
            eps128 = sing.tile([128, 1], F32)
            nc.vector.memset(eps128, EPS)
            ones_row = sing.tile([1, BC], BF16)
            nc.vector.memset(ones_row, 1.0)

            # ================= Phase T: trunk =================
            scope_T, _ = nc.enter_named_scope("phaseT", False)
            with (
                tc.tile_pool(name="t_pv", bufs=3, space="PSUM") as ps_pv,
                tc.tile_pool(name="t_ssr", bufs=2, space="PSUM") as ps_ssr,
                tc.tile_pool(name="t_lnt", bufs=2, space="PSUM") as ps_lnt,
            ):
                for j in range(32):
                    ta, tb = j, 63 - j
                    oa = io_p.tile([128, BC], BF16, tag="obs")
                    nc.sync.dma_start(out=oa, in_=obsT[:, ta * BC : (ta + 1) * BC])
                    ob = io_p.tile([128, BC], BF16, tag="obs")
                    nc.sync.dma_start(out=ob, in_=obsT[:, tb * BC : (tb + 1) * BC])
                    at = io_p.tile([64, BC], BF16, tag="act")
                    nc.sync.dma_start(out=at[0:32, :], in_=actT[:, ta * BC : (ta + 1) * BC])
                    nc.sync.dma_start(out=at[32:64, :], in_=actT[:, tb * BC : (tb + 1) * BC])

                    def lnorm(pv, lidx):
                        """Replicated-ss LayerNorm: returns rstd [128,BC] f32 SBUF."""
                        vsq = big.tile([128, BC], BF16, name=f"vsq{lidx}", tag="vsq")
                        if (lidx == 0 and consts["b1_nz"]) or (lidx == 1 and consts["b2_nz"]):
                            nc.scalar.activation(vsq, pv, AF.Square, bias=ct["bstk"][:, lidx : lidx + 1])
                        else:
                            nc.scalar.activation(vsq, pv, AF.Square)
                        ssr = ps_ssr.tile([128, BC], F32, name=f"ssr{lidx}", tag="ssr")
                        nc.tensor.matmul(ssr, ct["REP"], vsq, start=True, stop=True)
                        lnt = ps_lnt.tile([128, BC], F32, name=f"lnt{lidx}", tag="lnt")
                        nc.scalar.activation(lnt, ssr, AF.Ln, bias=eps128, scale=1.0 / 64.0)
                        rstd = big.tile([128, BC], F32, name=f"rstd{lidx}", tag="rstd")
                        nc.scalar.activation(rstd, lnt, AF.Exp, scale=-0.5)
                        return rstd

                    def relu_norm(out_ap, pv, rstd, lidx):
                        triv = consts["g1_triv"] if lidx == 0 else consts["g2_triv"]
                        gbe = ct["g1be1"] if lidx == 0 else ct["g2be2"]
                        bias_nz = consts["b1_nz"] if lidx == 0 else consts["b2_nz"]
                        if triv and not bias_nz:
                            nc.vector.scalar_tensor_tensor(
                                out=out_ap, in0=pv, scalar=0.0, in1=rstd,
                                op0=ALU.max, op1=ALU.mult,
                            )
                        else:
                            t1 = big.tile([128, BC], F32, name=f"t1_{lidx}", tag="t1")
                            nc.vector.scalar_tensor_tensor(
                                out=t1, in0=pv,
                                scalar=ct["bstk"][:, lidx : lidx + 1] if bias_nz else 0.0,
                                in1=rstd,
                                op0=ALU.add if bias_nz else ALU.max, op1=ALU.mult,
                            )
                            nc.scalar.activation(
                                out_ap, t1, AF.Relu,
                                bias=gbe[:, 1:2], scale=gbe[:, 0:1],
                            )

                    pv = ps_pv.tile([128, BC], F32, tag="pv")
                    nc.tensor.matmul(pv[0:64, :], ct["W1c"], oa, start=True, stop=True,
                                     tile_position=(0, 0))
                    nc.tensor.matmul(pv[64:128, :], ct["W1c"], ob, start=True, stop=True,
                                     tile_position=(0, 64))
                    rstd1 = lnorm(pv, 0)
                    xn = big.tile([128, BC], BF16, tag="xn")
                    relu_norm(xn, pv, rstd1, 0)

                    pv2 = ps_pv.tile([128, BC], F32, name="pv2", tag="pv")
                    nc.tensor.matmul(pv2, ct["W2X"], xn, start=True, stop=False)
                    nc.tensor.matmul(pv2, ct["W2A"], at, start=False, stop=True)
                    rstd2 = lnorm(pv2, 1)
                    relu_norm(X2[:, j * BC : (j + 1) * BC], pv2, rstd2, 1)

            nc.leave_named_scope("phaseT", scope_T, False)
            tc.strict_bb_all_engine_barrier()

            # ================= Phase L: LSTM =================
            scope_L, _ = nc.enter_named_scope("phaseL", False)
            xh_f = sing.tile([128, BC], BF16)
            xh_b = sing.tile([128, BC], BF16)
            cst = sing.tile([128, BC], F32)
            nc.vector.memset(xh_f[0:64, :], 0.0)
            nc.vector.memset(xh_b[0:64, :], 0.0)
            nc.vector.memset(cst, 0.0)
            S = sing.tile([128, 4 * BC], BF16)
            tJ = sing.tile([128, BC], BF16)
            u = sing.tile([128, BC], BF16)
            cf = sing.tile([128, BC], F32)
            tcl = sing.tile([128, BC], BF16)

            lstm_ps_cm = tc.tile_pool(name="lstm_ps", bufs=1, space="PSUM")
            lstm_ps = lstm_ps_cm.__enter__()
            PG = lstm_ps.tile([128, 4 * BC], F32, tag="PG")
            ph4 = [lstm_ps.tile([4, BC], F32, name=f"ph4_{i}", tag=f"ph4_{i}") for i in range(2)]
            stg = [sing.tile([4, BC], BF16, name=f"stg{i}", tag=f"stg{i}") for i in range(2)]

            def xsrc(t_needed):
                if t_needed < 32:
                    return X2[0:64, t_needed * BC : (t_needed + 1) * BC]
                jj = 63 - t_needed
                return X2[64:128, jj * BC : (jj + 1) * BC]

            for t in range(T):
                nc.gpsimd.tensor_copy(xh_f[64:128, :], xsrc(t))
                nc.gpsimd.tensor_copy(xh_b[64:128, :], xsrc(63 - t))
                for g in range(4):
                    sl = slice(g * BC, (g + 1) * BC)
                    bias_nz = consts["bias_nz"][g]
                    nc.tensor.matmul(PG[0:64, sl], ct[f"Wgf{g}"], xh_f, start=True,
                                     stop=not bias_nz, tile_position=(0, 0))
                    nc.tensor.matmul(PG[64:128, sl], ct[f"Wgb{g}"], xh_b, start=True,
                                     stop=not bias_nz, tile_position=(0, 64))
                    if bias_nz:
                        nc.tensor.matmul(PG[:, sl], ct[f"bias{g}"], ones_row,
                                         start=False, stop=True)
                nc.scalar.activation(S, PG, AF.Sigmoid)
                # u = sig_i * (2*sig_j - 1) = 2*sig_i*sig_j - sig_i
                nc.vector.tensor_tensor(out=tJ, in0=S[:, 0:BC], in1=S[:, BC : 2 * BC],
                                        op=ALU.mult)
                nc.vector.scalar_tensor_tensor(out=u, in0=tJ, scalar=2.0,
                                               in1=S[:, 0:BC], op0=ALU.mult,
                                               op1=ALU.subtract)
                nc.gpsimd.tensor_tensor(out=cf, in0=cst, in1=S[:, 2 * BC : 3 * BC], op=ALU.mult)
                nc.vector.tensor_tensor(out=cst, in0=cf, in1=u, op=ALU.add)
                nc.scalar.activation(tcl, cst, AF.Tanh)
                nc.vector.tensor_tensor(out=xh_f[0:64, :], in0=tcl[0:64, :],
                                        in1=S[0:64, 3 * BC : 4 * BC], op=ALU.mult)
                nc.vector.tensor_tensor(out=xh_b[0:64, :], in0=tcl[64:128, :],
                                        in1=S[64:128, 3 * BC : 4 * BC], op=ALU.mult)
                ph = ph4[t % 2]
                nc.tensor.matmul(ph, ct["W4F"], xh_f[0:64, :], start=True, stop=False)
                nc.tensor.matmul(ph, ct["W4B"], xh_b[0:64, :], start=False, stop=True)
                sg = stg[t % 2]
                nc.vector.tensor_copy(sg, ph)
                nc.sync.dma_start(out=xs_dram[:, t * BC : (t + 1) * BC], in_=sg)

            lstm_ps_cm.__exit__(None, None, None)
            nc.leave_named_scope("phaseL", scope_L, False)
            tc.strict_bb_all_engine_barrier()

            # ================= Phase H: head =================
            scope_H, _ = nc.enter_named_scope("phaseH", False)
            head_ps_cm = tc.tile_pool(name="head_ps", bufs=1, space="PSUM")
            ps_h = head_ps_cm.__enter__()
            eps64 = sing.tile([64, 1], F32)
            nc.vector.memset(eps64, EPS)
            ovs = [sing.tile([1, BC], F32, name=f"ovs{d}", tag=f"ovs{d}") for d in range(2)]
            for d, (w3, b3, gpb) in enumerate(
                (("W3f", "b3f", "gpbepf"), ("W3b", "b3b", "gpbepb"))
            ):
                xsT = big.tile([64, BC], BF16, name=f"xsT{d}", tag="hxs")
                psT = big.tile([64, BC], BF16, name=f"psT{d}", tag="hps")
                nc.sync.dma_start(
                    out=xsT,
                    in_=xs_dram[2 * d : 2 * d + 1, :].rearrange("o (t b) -> (o t) b", b=BC),
                )
                nc.sync.dma_start(
                    out=psT,
                    in_=xs_dram[2 * d + 1 : 2 * d + 2, :].rearrange("o (t b) -> (o t) b", b=BC),
                )
                pc = ps_h.tile([64, BC], F32, name=f"pc{d}", tag="hpc")
                nc.tensor.matmul(pc, ct["CT"], psT, start=True, stop=True)
                hsq = big.tile([64, BC], BF16, name=f"hsq{d}", tag="vsq")
                nc.scalar.activation(hsq, pc, AF.Square)
                ssrh = ps_h.tile([64, BC], F32, name=f"ssrh{d}", tag="hssr")
                nc.tensor.matmul(ssrh, ct["REP64"], hsq, start=True, stop=True)
                lnh = ps_h.tile([64, BC], F32, name=f"lnh{d}", tag="hlnt")
                nc.scalar.activation(lnh, ssrh, AF.Ln, bias=eps64, scale=1.0 / 64.0)
                preph = big.tile([64, BC], F32, name=f"preph{d}", tag="rstd")
                nc.scalar.activation(preph, lnh, AF.Exp, scale=-0.5)
                pn = big.tile([64, BC], BF16, name=f"pn{d}", tag="xn")
                if consts["gp_triv"]:
                    nc.vector.scalar_tensor_tensor(
                        out=pn, in0=pc, scalar=0.0, in1=preph, op0=ALU.max, op1=ALU.mult
                    )
                else:
                    t1 = big.tile([64, BC], F32, name=f"ht1{d}", tag="t1")
                    nc.vector.scalar_tensor_tensor(
                        out=t1, in0=pc, scalar=1.0, in1=preph, op0=ALU.mult, op1=ALU.mult
                    )
                    nc.scalar.activation(pn, t1, AF.Relu, bias=ct[gpb][:, 1:2],
                                         scale=ct[gpb][:, 0:1])
                pl = ps_h.tile([64, BC], F32, name=f"pl{d}", tag="hpl")
                nc.tensor.matmul(pl, ct[w3], pn, start=True, stop=True)
                he = big.tile([64, BC], BF16, name=f"he{d}", tag="he")
                nc.scalar.activation(he, pl, AF.Exp, bias=ct[b3][:, 0:1])
                hse = ps_h.tile([1, BC], F32, name=f"hse{d}", tag="hse")
                nc.tensor.matmul(hse, ct["ones64"], he, start=True, stop=True)
                lnse = ps_h.tile([1, BC], F32, name=f"lnse{d}", tag="hlns")
                nc.scalar.activation(lnse, hse, AF.Ln)
                hrs = big.tile([1, BC], F32, name=f"hrs{d}", tag="hrs")
                nc.scalar.activation(hrs, lnse, AF.Exp, scale=-1.0)
                hex_ = big.tile([64, BC], BF16, name=f"hex{d}", tag="hex")
                nc.vector.tensor_tensor(out=hex_, in0=he, in1=xsT, op=ALU.mult)
                hnum = ps_h.tile([1, BC], F32, name=f"hnum{d}", tag="hnum")
                nc.tensor.matmul(hnum, ct["ones64"], hex_, start=True, stop=True)
                hov = big.tile([1, BC], F32, name=f"hov{d}", tag="hov")
                nc.vector.tensor_tensor(out=hov, in0=hnum, in1=hrs, op=ALU.mult)
                nc.vector.tensor_scalar(out=ovs[d], in0=hov, scalar1=float(consts["bx"]),
                                        scalar2=None, op0=ALU.add)
                nc.sync.dma_start(out=ov[d : d + 1, :], in_=ovs[d])
            head_ps_cm.__exit__(None, None, None)
            nc.leave_named_scope("phaseH", scope_H, False)

    return nc


_CACHE = {}


def kernel(**inputs):
    obs = np.ascontiguousarray(inputs["obs"], dtype=np.float32)
    action = np.ascontiguousarray(inputs["action"], dtype=np.float32)
    consts = _prep_consts(inputs)

    key = "nc"
    if key not in _CACHE:
        _CACHE[key] = _install_waitfix(_build(consts))
    nc = _CACHE[key]

    const_feed = {}
    for name in _BF16_CONSTS + _F32_CONSTS:
        const_feed[name] = consts[name]

    in_maps = []
    for c in range(NCORES):
        sl = slice(c * BC * T, (c + 1) * BC * T)
        obsT = np.ascontiguousarray(
            obs[sl].reshape(BC, T, OBS).transpose(2, 1, 0).reshape(OBS, NT)
        ).astype(BF)
        actTm = np.ascontiguousarray(
            action[sl].reshape(BC, T, ACTD).transpose(2, 1, 0).reshape(ACTD, NT)
        ).astype(BF)
        m = {"obsT": obsT, "actT": actTm}
        m.update(const_feed)
        in_maps.append(m)

    kw = {}
    if os.environ.get("BASS_TRACE"):
        kw["trace"] = True
        if os.environ.get("BASS_TRACE_DIR"):
            kw["tmpdir"] = os.environ["BASS_TRACE_DIR"]
    full = run_bass_kernel_spmd(nc, in_maps, list(range(NCORES)), **kw)
    globals()["LAST_RESULTS"] = full
    res = full.results

    out = np.empty(2 * B, dtype=np.float32)
    for c in range(NCORES):
        ovc = res[c]["ov"]
        out[c * BC : (c + 1) * BC] = ovc[0]
        out[B + c * BC : B + (c + 1) * BC] = ovc[1]
    return out


# revision 17
# speedup vs baseline: 1.9203x; 1.2468x over previous
"""Trainium2 Bass kernel for nn_Critic (bidirectional-LSTM critic network).

v2: all matmuls bf16 (1 cyc/row vs fp32's 4), LayerNorm via replicated
sum-of-squares matmul + Ln/Exp rsqrt (kills the 3.3us InstReciprocal),
relu+normalize fused into one DVE scalar_tensor_tensor, LSTM gates as a
single fused Sigmoid over a 4-bank PSUM tile (tanh(j) = 2*sigmoid(2j)-1
folded into weights), forget bias via K=1 accumulating matmul, cell-state
mul offloaded to GpSimd, head stash in bf16 with softmax reciprocal as
exp(-ln(x)).

Data-parallel over B: 8 NeuronCores x 512 sequences. Feature-major layout;
trunk processes timestep pairs (j, 63-j) packed on partitions [ta|tb].
"""

import os
import sys

for _p in ("/opt/trn_rl_repo",):
    if _p not in sys.path:
        sys.path.insert(0, _p)

import json as _json
from types import MethodType as _MethodType

import ml_dtypes
import numpy as np

import concourse.bass as bass
import concourse.tile as tile
from concourse import mybir
from concourse.bass_utils import run_bass_kernel_spmd

F32 = mybir.dt.float32
BF16 = mybir.dt.bfloat16
AF = mybir.ActivationFunctionType
ALU = mybir.AluOpType
BF = ml_dtypes.bfloat16

B, T, H, OBS, ACTD = 4096, 64, 64, 128, 32
NCORES = 8
BC = B // NCORES  # 512 sequences per core
NT = T * BC
EPS = 1e-12

# ---------------------------------------------------------------- waitfix --
_MAX_WAITS = 1


def _patch_bir(bir):
    n = [0]

    def fresh():
        n[0] += 1
        return f"I-waitfix-{n[0]}"

    for fn in bir.get("functions", []):
        for bb in fn.get("blocks", []):
            out = []
            for inst in bb.get("instructions", []):
                si = inst.get("sync_info") or {}
                waits = si.get("on_wait") or []
                if len(waits) > _MAX_WAITS:
                    extra = waits[: len(waits) - _MAX_WAITS]
                    keep = waits[len(waits) - _MAX_WAITS :]
                    for i in range(0, len(extra), _MAX_WAITS):
                        out.append(
                            {
                                "name": fresh(),
                                "opcode": "NoOp",
                                "engine": inst["engine"],
                                "ins": [],
                                "outs": [],
                                "sync_info": {
                                    "on_wait": extra[i : i + _MAX_WAITS],
                                    "on_update": [],
                                },
                            }
                        )
                    si = dict(si)
                    si["on_wait"] = keep
                    inst = dict(inst)
                    inst["sync_info"] = si
                out.append(inst)
            bb["instructions"] = out
    return bir


def _install_waitfix(nc):
    orig = nc.to_json_bytes

    def patched(self):
        return _json.dumps(_patch_bir(_json.loads(orig()))).encode()

    nc.to_json_bytes = _MethodType(patched, nc)
    return nc


# ----------------------------------------------------------- host weights --


def _prep_consts(p):
    f32 = lambda x: np.ascontiguousarray(x, dtype=np.float32)
    bf = lambda x: np.ascontiguousarray(np.asarray(x, np.float64), dtype=np.float32).astype(BF)
    C = np.eye(64, dtype=np.float64) - 1.0 / 64.0

    c = {}
    c["W1c"] = bf(np.asarray(p["W1"], np.float64) @ C)  # [128, 64]
    W2c = np.asarray(p["W2"], np.float64) @ C  # [96, 64]
    blk = np.zeros((128, 128), np.float64)
    blk[0:64, 0:64] = W2c[0:64]
    blk[64:128, 64:128] = W2c[0:64]
    c["W2X"] = bf(blk)
    blka = np.zeros((64, 128), np.float64)
    blka[0:32, 0:64] = W2c[64:96]
    blka[32:64, 64:128] = W2c[64:96]
    c["W2A"] = bf(blka)
    rep = np.zeros((128, 128), np.float64)
    rep[0:64, 0:64] = 1.0
    rep[64:128, 64:128] = 1.0
    c["REP"] = bf(rep)

    # trunk fast-path flags and general-path params
    c["b1_nz"] = bool(np.any(np.asarray(p["b1"])))
    c["b2_nz"] = bool(np.any(np.asarray(p["b2"])))
    b1c = np.asarray(p["b1"], np.float64) @ C
    b2c = np.asarray(p["b2"], np.float64) @ C
    c["bstk"] = f32(np.stack([np.tile(b1c, 2), np.tile(b2c, 2)], 1))  # [128,2]
    c["g1_triv"] = bool(
        np.all(np.asarray(p["g1"]) == 1) and np.all(np.asarray(p["be1"]) == 0)
    )
    c["g2_triv"] = bool(
        np.all(np.asarray(p["g2"]) == 1) and np.all(np.asarray(p["be2"]) == 0)
    )
    c["g1be1"] = f32(np.stack([np.tile(p["g1"], 2), np.tile(p["be1"], 2)], 1))
    c["g2be2"] = f32(np.stack([np.tile(p["g2"], 2), np.tile(p["be2"], 2)], 1))

    # LSTM gate weights: lhsT rows = [h (64); x (64)].
    # PG bank order (i, f, o, j): slots 0..2 share one fused Sigmoid,
    # slot 3 gets its own Tanh.
    Wf = np.asarray(p["Wf"], np.float64)
    Wb = np.asarray(p["Wb"], np.float64)
    bfv = np.asarray(p["bf"], np.float64).copy()
    bbv = np.asarray(p["bb"], np.float64).copy()
    bfv[128:192] += 1.0  # forget_bias on f gate
    bbv[128:192] += 1.0
    gb = []
    for slot, g in enumerate((0, 2, 3, 1)):  # ref gates i,j,f,o -> slots i,f,o,j
        sl = slice(g * 64, (g + 1) * 64)
        wgf = np.concatenate([Wf[64:128, sl], Wf[0:64, sl]], 0)
        wgb = np.concatenate([Wb[64:128, sl], Wb[0:64, sl]], 0)
        c[f"Wgf{slot}"] = bf(wgf)
        c[f"Wgb{slot}"] = bf(wgb)
        gb.append(np.concatenate([bfv[sl], bbv[sl]]))
    c["bias_nz"] = [bool(np.any(gb[s])) for s in range(4)]
    for s in range(4):
        c[f"bias{s}"] = bf(gb[s].reshape(1, 128))

    wx = np.asarray(p["wx"], np.float64)
    wp = np.asarray(p["wp"], np.float64)
    w4f = np.zeros((64, 4), np.float64)
    w4f[:, 0] = wx
    w4f[:, 1] = wp
    w4b = np.zeros((64, 4), np.float64)
    w4b[:, 2] = wx
    w4b[:, 3] = wp
    c["W4F"] = bf(w4f)
    c["W4B"] = bf(w4b)

    c["CT"] = bf(np.eye(64) - 1.0 / 64.0)
    c["REP64"] = bf(np.ones((64, 64)))
    c["ones64"] = bf(np.ones((64, 1)))
    c["W3f"] = bf(p["W3"])
    c["W3b"] = bf(np.asarray(p["W3"])[::-1, ::-1])
    c["b3f"] = f32(np.asarray(p["b3"]).reshape(64, 1))
    c["b3b"] = f32(np.asarray(p["b3"])[::-1].reshape(64, 1))
    c["gp_triv"] = bool(
        np.all(np.asarray(p["gp"]) == 1) and np.all(np.asarray(p["bep"]) == 0)
    )
    c["gpbepf"] = f32(np.stack([p["gp"], p["bep"]], 1))
    c["gpbepb"] = f32(np.stack([np.asarray(p["gp"])[::-1], np.asarray(p["bep"])[::-1]], 1))
    c["bx"] = float(np.asarray(p["bx"]))
    return c


# ------------------------------------------------------------ bass program --

_BF16_CONSTS = (
    ["W1c", "W2X", "W2A", "REP", "W4F", "W4B", "CT", "REP64", "ones64", "W3f", "W3b"]
    + [f"Wgf{g}" for g in range(4)]
    + [f"Wgb{g}" for g in range(4)]
    + [f"bias{g}" for g in range(4)]
)
_F32_CONSTS = ["bstk", "g1be1", "g2be2", "b3f", "b3b", "gpbepf", "gpbepb"]


def _build(consts):
    nc = bass.Bass()
    obsT = nc.declare_dram_parameter("obsT", [128, NT], BF16, isOutput=False)
    actT = nc.declare_dram_parameter("actT", [32, NT], BF16, isOutput=False)

    cin = {}
    for name in _BF16_CONSTS:
        cin[name] = nc.declare_dram_parameter(name, list(consts[name].shape), BF16, isOutput=False)
    for name in _F32_CONSTS:
        cin[name] = nc.declare_dram_parameter(name, list(consts[name].shape), F32, isOutput=False)

    ov = nc.declare_dram_parameter("ov", [2, BC], F32, isOutput=True)
    xs_dram = nc.dram_tensor("xs_stash", [4, NT], BF16)

    with tile.TileContext(nc) as tc:
        with (
            tc.tile_pool(name="singles", bufs=1) as sing,
            tc.tile_pool(name="io_p", bufs=3) as io_p,
            tc.tile_pool(name="big", bufs=2) as big,
        ):
            ct = {}
            for name in _BF16_CONSTS + _F32_CONSTS:
                ct[name] = sing.tile(
                    list(consts[name].shape),
                    BF16 if name in _BF16_CONSTS else F32,
                    name=f"ct_{name}",
                    tag=f"ct_{name}",
                )
                nc.sync.dma_start(out=ct[name], in_=cin[name][:, :])

            X2 = sing.tile([128, 32 * BC], BF16)
            eps128 = sing.tile([128, 1], F32)
            nc.vector.memset(eps128, EPS)
            ones_row = sing.tile([1, BC], BF16)
            nc.vector.memset(ones_row, 1.0)

            # ================= Phase T: trunk =================
            scope_T, _ = nc.enter_named_scope("phaseT", False)
            with (
                tc.tile_pool(name="t_pv", bufs=3, space="PSUM") as ps_pv,
                tc.tile_pool(name="t_ssr", bufs=2, space="PSUM") as ps_ssr,
                tc.tile_pool(name="t_lnt", bufs=2, space="PSUM") as ps_lnt,
            ):
                for j in range(32):
                    ta, tb = j, 63 - j
                    oa = io_p.tile([128, BC], BF16, tag="obs")
                    nc.sync.dma_start(out=oa, in_=obsT[:, ta * BC : (ta + 1) * BC])
                    ob = io_p.tile([128, BC], BF16, tag="obs")
                    nc.sync.dma_start(out=ob, in_=obsT[:, tb * BC : (tb + 1) * BC])
                    at = io_p.tile([64, BC], BF16, tag="act")
                    nc.sync.dma_start(out=at[0:32, :], in_=actT[:, ta * BC : (ta + 1) * BC])
                    nc.sync.dma_start(out=at[32:64, :], in_=actT[:, tb * BC : (tb + 1) * BC])

                    def lnorm(pv, lidx):
                        """Replicated-ss LayerNorm: returns rstd [128,BC] f32 SBUF."""
                        vsq = big.tile([128, BC], BF16, name=f"vsq{lidx}", tag="vsq")
                        if (lidx == 0 and consts["b1_nz"]) or (lidx == 1 and consts["b2_nz"]):
                            nc.scalar.activation(vsq, pv, AF.Square, bias=ct["bstk"][:, lidx : lidx + 1])
                        else:
                            nc.scalar.activation(vsq, pv, AF.Square)
                        ssr = ps_ssr.tile([128, BC], F32, name=f"ssr{lidx}", tag="ssr")
                        nc.tensor.matmul(ssr, ct["REP"], vsq, start=True, stop=True)
                        lnt = ps_lnt.tile([128, BC], F32, name=f"lnt{lidx}", tag="lnt")
                        nc.scalar.activation(lnt, ssr, AF.Ln, bias=eps128, scale=1.0 / 64.0)
                        rstd = big.tile([128, BC], F32, name=f"rstd{lidx}", tag="rstd")
                        nc.scalar.activation(rstd, lnt, AF.Exp, scale=-0.5)
                        return rstd

                    def relu_norm(out_ap, pv, rstd, lidx):
                        triv = consts["g1_triv"] if lidx == 0 else consts["g2_triv"]
                        gbe = ct["g1be1"] if lidx == 0 else ct["g2be2"]
                        bias_nz = consts["b1_nz"] if lidx == 0 else consts["b2_nz"]
                        if triv and not bias_nz:
                            nc.vector.scalar_tensor_tensor(
                                out=out_ap, in0=pv, scalar=0.0, in1=rstd,
                                op0=ALU.max, op1=ALU.mult,
                            )
                        else:
                            t1 = big.tile([128, BC], F32, name=f"t1_{lidx}", tag="t1")
                            nc.vector.scalar_tensor_tensor(
                                out=t1, in0=pv,
                                scalar=ct["bstk"][:, lidx : lidx + 1] if bias_nz else 0.0,
                                in1=rstd,
                                op0=ALU.add if bias_nz else ALU.max, op1=ALU.mult,
                            )
                            nc.scalar.activation(
                                out_ap, t1, AF.Relu,
                                bias=gbe[:, 1:2], scale=gbe[:, 0:1],
                            )

                    pv = ps_pv.tile([128, BC], F32, tag="pv")
                    nc.tensor.matmul(pv[0:64, :], ct["W1c"], oa, start=True, stop=True,
                                     tile_position=(0, 0))
                    nc.tensor.matmul(pv[64:128, :], ct["W1c"], ob, start=True, stop=True,
                                     tile_position=(0, 64))
                    rstd1 = lnorm(pv, 0)
                    xn = big.tile([128, BC], BF16, tag="xn")
                    relu_norm(xn, pv, rstd1, 0)

                    pv2 = ps_pv.tile([128, BC], F32, name="pv2", tag="pv")
                    nc.tensor.matmul(pv2, ct["W2X"], xn, start=True, stop=False)
                    nc.tensor.matmul(pv2, ct["W2A"], at, start=False, stop=True)
                    rstd2 = lnorm(pv2, 1)
                    relu_norm(X2[:, j * BC : (j + 1) * BC], pv2, rstd2, 1)

            nc.leave_named_scope("phaseT", scope_T, False)
            tc.strict_bb_all_engine_barrier()

            # ================= Phase L: LSTM =================
            scope_L, _ = nc.enter_named_scope("phaseL", False)
            xh_f = sing.tile([128, BC], BF16)
            xh_b = sing.tile([128, BC], BF16)
            cst = sing.tile([128, BC], F32)
            nc.vector.memset(xh_f[0:64, :], 0.0)
            nc.vector.memset(xh_b[0:64, :], 0.0)
            nc.vector.memset(cst, 0.0)
            S = sing.tile([128, 3 * BC], BF16)  # sigmoid(i, f, o)
            tJ = sing.tile([128, BC], BF16)  # tanh(j)
            u = sing.tile([128, BC], BF16)
            cf = sing.tile([128, BC], F32)
            tcl = sing.tile([128, BC], BF16)

            lstm_ps_cm = tc.tile_pool(name="lstm_ps", bufs=1, space="PSUM")
            lstm_ps = lstm_ps_cm.__enter__()
            PG = lstm_ps.tile([128, 4 * BC], F32, tag="PG")
            ph4 = [lstm_ps.tile([4, BC], F32, name=f"ph4_{i}", tag=f"ph4_{i}") for i in range(2)]
            stg = [sing.tile([4, BC], BF16, name=f"stg{i}", tag=f"stg{i}") for i in range(2)]

            def xsrc(t_needed):
                if t_needed < 32:
                    return X2[0:64, t_needed * BC : (t_needed + 1) * BC]
                jj = 63 - t_needed
                return X2[64:128, jj * BC : (jj + 1) * BC]

            def head_dots(t):
                ph = ph4[t % 2]
                nc.tensor.matmul(ph, ct["W4F"], xh_f[0:64, :], start=True, stop=False)
                nc.tensor.matmul(ph, ct["W4B"], xh_b[0:64, :], start=False, stop=True)
                sg = stg[t % 2]
                nc.vector.tensor_copy(sg, ph)
                nc.sync.dma_start(out=xs_dram[:, t * BC : (t + 1) * BC], in_=sg)

            # prefetch x for t=0
            nc.sync.dma_start(out=xh_f[64:128, :], in_=xsrc(0))
            nc.sync.dma_start(out=xh_b[64:128, :], in_=xsrc(63))

            for t in range(T):
                for s in range(4):
                    sl = slice(s * BC, (s + 1) * BC)
                    bias_nz = consts["bias_nz"][s]
                    nc.tensor.matmul(PG[0:64, sl], ct[f"Wgf{s}"], xh_f, start=True,
                                     stop=not bias_nz, tile_position=(0, 0))
                    nc.tensor.matmul(PG[64:128, sl], ct[f"Wgb{s}"], xh_b, start=True,
                                     stop=not bias_nz, tile_position=(0, 64))
                    if bias_nz:
                        nc.tensor.matmul(PG[:, sl], ct[f"bias{s}"], ones_row,
                                         start=False, stop=True)
                # head dots of previous step run off the critical path (PE is
                # idle during sigma/cell ops); they read h(t-1) before the
                # h-mul below overwrites it (Tile WAR dep).
                if t > 0:
                    head_dots(t - 1)
                # x prefetch for t+1 via DMA, after this step's gate matmuls
                if t + 1 < T:
                    nc.sync.dma_start(out=xh_f[64:128, :], in_=xsrc(t + 1))
                    nc.sync.dma_start(out=xh_b[64:128, :], in_=xsrc(62 - t))
                nc.scalar.activation(S, PG[:, 0 : 3 * BC], AF.Sigmoid)
                nc.scalar.activation(tJ, PG[:, 3 * BC : 4 * BC], AF.Tanh)
                nc.gpsimd.tensor_tensor(out=cf, in0=cst, in1=S[:, BC : 2 * BC], op=ALU.mult)
                nc.vector.tensor_tensor(out=u, in0=S[:, 0:BC], in1=tJ, op=ALU.mult)
                nc.vector.tensor_tensor(out=cst, in0=cf, in1=u, op=ALU.add)
                nc.scalar.activation(tcl, cst, AF.Tanh)
                nc.vector.tensor_tensor(out=xh_f[0:64, :], in0=tcl[0:64, :],
                                        in1=S[0:64, 2 * BC : 3 * BC], op=ALU.mult)
                nc.vector.tensor_tensor(out=xh_b[0:64, :], in0=tcl[64:128, :],
                                        in1=S[64:128, 2 * BC : 3 * BC], op=ALU.mult)
            head_dots(T - 1)

            lstm_ps_cm.__exit__(None, None, None)
            nc.leave_named_scope("phaseL", scope_L, False)
            tc.strict_bb_all_engine_barrier()

            # ================= Phase H: head =================
            scope_H, _ = nc.enter_named_scope("phaseH", False)
            head_ps_cm = tc.tile_pool(name="head_ps", bufs=1, space="PSUM")
            ps_h = head_ps_cm.__enter__()
            eps64 = sing.tile([64, 1], F32)
            nc.vector.memset(eps64, EPS)
            ovs = [sing.tile([1, BC], F32, name=f"ovs{d}", tag=f"ovs{d}") for d in range(2)]
            for d, (w3, b3, gpb) in enumerate(
                (("W3f", "b3f", "gpbepf"), ("W3b", "b3b", "gpbepb"))
            ):
                xsT = big.tile([64, BC], BF16, name=f"xsT{d}", tag="hxs")
                psT = big.tile([64, BC], BF16, name=f"psT{d}", tag="hps")
                nc.sync.dma_start(
                    out=xsT,
                    in_=xs_dram[2 * d : 2 * d + 1, :].rearrange("o (t b) -> (o t) b", b=BC),
                )
                nc.sync.dma_start(
                    out=psT,
                    in_=xs_dram[2 * d + 1 : 2 * d + 2, :].rearrange("o (t b) -> (o t) b", b=BC),
                )
                pc = ps_h.tile([64, BC], F32, name=f"pc{d}", tag="hpc")
                nc.tensor.matmul(pc, ct["CT"], psT, start=True, stop=True)
                hsq = big.tile([64, BC], BF16, name=f"hsq{d}", tag="vsq")
                nc.scalar.activation(hsq, pc, AF.Square)
                ssrh = ps_h.tile([64, BC], F32, name=f"ssrh{d}", tag="hssr")
                nc.tensor.matmul(ssrh, ct["REP64"], hsq, start=True, stop=True)
                lnh = ps_h.tile([64, BC], F32, name=f"lnh{d}", tag="hlnt")
                nc.scalar.activation(lnh, ssrh, AF.Ln, bias=eps64, scale=1.0 / 64.0)
                preph = big.tile([64, BC], F32, name=f"preph{d}", tag="rstd")
                nc.scalar.activation(preph, lnh, AF.Exp, scale=-0.5)
                pn = big.tile([64, BC], BF16, name=f"pn{d}", tag="xn")
                if consts["gp_triv"]:
                    nc.vector.scalar_tensor_tensor(
                        out=pn, in0=pc, scalar=0.0, in1=preph, op0=ALU.max, op1=ALU.mult
                    )
                else:
                    t1 = big.tile([64, BC], F32, name=f"ht1{d}", tag="t1")
                    nc.vector.scalar_tensor_tensor(
                        out=t1, in0=pc, scalar=1.0, in1=preph, op0=ALU.mult, op1=ALU.mult
                    )
                    nc.scalar.activation(pn, t1, AF.Relu, bias=ct[gpb][:, 1:2],
                                         scale=ct[gpb][:, 0:1])
                pl = ps_h.tile([64, BC], F32, name=f"pl{d}", tag="hpl")
                nc.tensor.matmul(pl, ct[w3], pn, start=True, stop=True)
                he = big.tile([64, BC], BF16, name=f"he{d}", tag="he")
                nc.scalar.activation(he, pl, AF.Exp, bias=ct[b3][:, 0:1])
                hse = ps_h.tile([1, BC], F32, name=f"hse{d}", tag="hse")
                nc.tensor.matmul(hse, ct["ones64"], he, start=True, stop=True)
                lnse = ps_h.tile([1, BC], F32, name=f"lnse{d}", tag="hlns")
                nc.scalar.activation(lnse, hse, AF.Ln)
                hrs = big.tile([1, BC], F32, name=f"hrs{d}", tag="hrs")
                nc.scalar.activation(hrs, lnse, AF.Exp, scale=-1.0)
                hex_ = big.tile([64, BC], BF16, name=f"hex{d}", tag="hex")
                nc.vector.tensor_tensor(out=hex_, in0=he, in1=xsT, op=ALU.mult)
                hnum = ps_h.tile([1, BC], F32, name=f"hnum{d}", tag="hnum")
                nc.tensor.matmul(hnum, ct["ones64"], hex_, start=True, stop=True)
                hov = big.tile([1, BC], F32, name=f"hov{d}", tag="hov")
                nc.vector.tensor_tensor(out=hov, in0=hnum, in1=hrs, op=ALU.mult)
                nc.vector.tensor_scalar(out=ovs[d], in0=hov, scalar1=float(consts["bx"]),
                                        scalar2=None, op0=ALU.add)
                nc.sync.dma_start(out=ov[d : d + 1, :], in_=ovs[d])
            head_ps_cm.__exit__(None, None, None)
            nc.leave_named_scope("phaseH", scope_H, False)

    return nc


_CACHE = {}


def kernel(**inputs):
    obs = np.ascontiguousarray(inputs["obs"], dtype=np.float32)
    action = np.ascontiguousarray(inputs["action"], dtype=np.float32)
    consts = _prep_consts(inputs)

    key = "nc"
    if key not in _CACHE:
        _CACHE[key] = _install_waitfix(_build(consts))
    nc = _CACHE[key]

    const_feed = {}
    for name in _BF16_CONSTS + _F32_CONSTS:
        const_feed[name] = consts[name]

    in_maps = []
    for c in range(NCORES):
        sl = slice(c * BC * T, (c + 1) * BC * T)
        obsT = np.ascontiguousarray(
            obs[sl].reshape(BC, T, OBS).transpose(2, 1, 0).reshape(OBS, NT)
        ).astype(BF)
        actTm = np.ascontiguousarray(
            action[sl].reshape(BC, T, ACTD).transpose(2, 1, 0).reshape(ACTD, NT)
        ).astype(BF)
        m = {"obsT": obsT, "actT": actTm}
        m.update(const_feed)
        in_maps.append(m)

    kw = {}
    if os.environ.get("BASS_TRACE"):
        kw["trace"] = True
        if os.environ.get("BASS_TRACE_DIR"):
            kw["tmpdir"] = os.environ["BASS_TRACE_DIR"]
    full = run_bass_kernel_spmd(nc, in_maps, list(range(NCORES)), **kw)
    globals()["LAST_RESULTS"] = full
    res = full.results

    out = np.empty(2 * B, dtype=np.float32)
    for c in range(NCORES):
        ovc = res[c]["ov"]
        out[c * BC : (c + 1) * BC] = ovc[0]
        out[B + c * BC : B + (c + 1) * BC] = ovc[1]
    return out


# revision 23
# speedup vs baseline: 2.0017x; 1.0424x over previous
"""Trainium2 Bass kernel for nn_Critic (bidirectional-LSTM critic network).

v2: all matmuls bf16 (1 cyc/row vs fp32's 4), LayerNorm via replicated
sum-of-squares matmul + Ln/Exp rsqrt (kills the 3.3us InstReciprocal),
relu+normalize fused into one DVE scalar_tensor_tensor, LSTM gates as a
single fused Sigmoid over a 4-bank PSUM tile (tanh(j) = 2*sigmoid(2j)-1
folded into weights), forget bias via K=1 accumulating matmul, cell-state
mul offloaded to GpSimd, head stash in bf16 with softmax reciprocal as
exp(-ln(x)).

Data-parallel over B: 8 NeuronCores x 512 sequences. Feature-major layout;
trunk processes timestep pairs (j, 63-j) packed on partitions [ta|tb].
"""

import os
import sys

for _p in ("/opt/trn_rl_repo",):
    if _p not in sys.path:
        sys.path.insert(0, _p)

import json as _json
from types import MethodType as _MethodType

import ml_dtypes
import numpy as np

import concourse.bass as bass
import concourse.tile as tile
from concourse import mybir
from concourse.bass_utils import run_bass_kernel_spmd

F32 = mybir.dt.float32
BF16 = mybir.dt.bfloat16
AF = mybir.ActivationFunctionType
ALU = mybir.AluOpType
BF = ml_dtypes.bfloat16

B, T, H, OBS, ACTD = 4096, 64, 64, 128, 32
NCORES = 8
BC = B // NCORES  # 512 sequences per core
NT = T * BC
EPS = 1e-12

# ---------------------------------------------------------------- waitfix --
_MAX_WAITS = 1


def _patch_bir(bir):
    n = [0]

    def fresh():
        n[0] += 1
        return f"I-waitfix-{n[0]}"

    for fn in bir.get("functions", []):
        for bb in fn.get("blocks", []):
            out = []
            for inst in bb.get("instructions", []):
                si = inst.get("sync_info") or {}
                waits = si.get("on_wait") or []
                if len(waits) > _MAX_WAITS:
                    extra = waits[: len(waits) - _MAX_WAITS]
                    keep = waits[len(waits) - _MAX_WAITS :]
                    for i in range(0, len(extra), _MAX_WAITS):
                        out.append(
                            {
                                "name": fresh(),
                                "opcode": "NoOp",
                                "engine": inst["engine"],
                                "ins": [],
                                "outs": [],
                                "sync_info": {
                                    "on_wait": extra[i : i + _MAX_WAITS],
                                    "on_update": [],
                                },
                            }
                        )
                    si = dict(si)
                    si["on_wait"] = keep
                    inst = dict(inst)
                    inst["sync_info"] = si
                out.append(inst)
            bb["instructions"] = out
    return bir


def _install_waitfix(nc):
    orig = nc.to_json_bytes

    def patched(self):
        return _json.dumps(_patch_bir(_json.loads(orig()))).encode()

    nc.to_json_bytes = _MethodType(patched, nc)
    return nc


# ----------------------------------------------------------- host weights --


def _prep_consts(p):
    f32 = lambda x: np.ascontiguousarray(x, dtype=np.float32)
    bf = lambda x: np.ascontiguousarray(np.asarray(x, np.float64), dtype=np.float32).astype(BF)
    C = np.eye(64, dtype=np.float64) - 1.0 / 64.0

    c = {}
    c["W1c"] = bf(np.asarray(p["W1"], np.float64) @ C)  # [128, 64]
    W2c = np.asarray(p["W2"], np.float64) @ C  # [96, 64]
    blk = np.zeros((128, 128), np.float64)
    blk[0:64, 0:64] = W2c[0:64]
    blk[64:128, 64:128] = W2c[0:64]
    c["W2X"] = bf(blk)
    blka = np.zeros((64, 128), np.float64)
    blka[0:32, 0:64] = W2c[64:96]
    blka[32:64, 64:128] = W2c[64:96]
    c["W2A"] = bf(blka)
    rep = np.zeros((128, 128), np.float64)
    rep[0:64, 0:64] = 1.0
    rep[64:128, 64:128] = 1.0
    c["REP"] = bf(rep)

    # trunk fast-path flags and general-path params
    c["b1_nz"] = bool(np.any(np.asarray(p["b1"])))
    c["b2_nz"] = bool(np.any(np.asarray(p["b2"])))
    b1c = np.asarray(p["b1"], np.float64) @ C
    b2c = np.asarray(p["b2"], np.float64) @ C
    c["bstk"] = f32(np.stack([np.tile(b1c, 2), np.tile(b2c, 2)], 1))  # [128,2]
    c["g1_triv"] = bool(
        np.all(np.asarray(p["g1"]) == 1) and np.all(np.asarray(p["be1"]) == 0)
    )
    c["g2_triv"] = bool(
        np.all(np.asarray(p["g2"]) == 1) and np.all(np.asarray(p["be2"]) == 0)
    )
    c["g1be1"] = f32(np.stack([np.tile(p["g1"], 2), np.tile(p["be1"], 2)], 1))
    c["g2be2"] = f32(np.stack([np.tile(p["g2"], 2), np.tile(p["be2"], 2)], 1))

    # LSTM gate weights: lhsT rows = [h (64); x (64)].
    # PG bank order (i, f, o, j): slots 0..2 share one fused Sigmoid,
    # slot 3 gets its own Tanh.
    Wf = np.asarray(p["Wf"], np.float64)
    Wb = np.asarray(p["Wb"], np.float64)
    bfv = np.asarray(p["bf"], np.float64).copy()
    bbv = np.asarray(p["bb"], np.float64).copy()
    bfv[128:192] += 1.0  # forget_bias on f gate
    bbv[128:192] += 1.0
    gb = []
    for slot, g in enumerate((2, 0, 3, 1)):  # slots f,i,o,j <- ref gates i,j,f,o
        sl = slice(g * 64, (g + 1) * 64)
        wgf = np.concatenate([Wf[64:128, sl], Wf[0:64, sl]], 0)
        wgb = np.concatenate([Wb[64:128, sl], Wb[0:64, sl]], 0)
        c[f"Wgf{slot}"] = bf(wgf)
        c[f"Wgb{slot}"] = bf(wgb)
        gb.append(np.concatenate([bfv[sl], bbv[sl]]))
    # slot 0 (f, incl forget_bias) is applied via sigma_f's per-partition
    # ACT bias; slots 1..3 get a K=1 bias matmul only if nonzero.
    c["fbias"] = f32(gb[0].reshape(128, 1))
    c["bias_nz"] = [False] + [bool(np.any(gb[s])) for s in range(1, 4)]
    for s in range(1, 4):
        c[f"bias{s}"] = bf(gb[s].reshape(1, 128))

    wx = np.asarray(p["wx"], np.float64)
    wp = np.asarray(p["wp"], np.float64)
    w4f = np.zeros((64, 4), np.float64)
    w4f[:, 0] = wx
    w4f[:, 1] = wp
    w4b = np.zeros((64, 4), np.float64)
    w4b[:, 2] = wx
    w4b[:, 3] = wp
    c["W4F"] = bf(w4f)
    c["W4B"] = bf(w4b)

    c["CT"] = bf(np.eye(64) - 1.0 / 64.0)
    c["REP64"] = bf(np.ones((64, 64)))
    c["ones64"] = bf(np.ones((64, 1)))
    c["W3f"] = bf(p["W3"])
    c["W3b"] = bf(np.asarray(p["W3"])[::-1, ::-1])
    c["b3f"] = f32(np.asarray(p["b3"]).reshape(64, 1))
    c["b3b"] = f32(np.asarray(p["b3"])[::-1].reshape(64, 1))
    c["gp_triv"] = bool(
        np.all(np.asarray(p["gp"]) == 1) and np.all(np.asarray(p["bep"]) == 0)
    )
    c["gpbepf"] = f32(np.stack([p["gp"], p["bep"]], 1))
    c["gpbepb"] = f32(np.stack([np.asarray(p["gp"])[::-1], np.asarray(p["bep"])[::-1]], 1))
    c["bx"] = float(np.asarray(p["bx"]))
    return c


# ------------------------------------------------------------ bass program --

_BF16_CONSTS = (
    ["W1c", "W2X", "W2A", "REP", "W4F", "W4B", "CT", "REP64", "ones64", "W3f", "W3b"]
    + [f"Wgf{g}" for g in range(4)]
    + [f"Wgb{g}" for g in range(4)]
    + [f"bias{g}" for g in range(1, 4)]
)
_F32_CONSTS = ["bstk", "g1be1", "g2be2", "b3f", "b3b", "gpbepf", "gpbepb", "fbias"]


def _build(consts):
    nc = bass.Bass()
    obsT = nc.declare_dram_parameter("obsT", [128, NT], BF16, isOutput=False)
    actT = nc.declare_dram_parameter("actT", [32, NT], BF16, isOutput=False)

    cin = {}
    for name in _BF16_CONSTS:
        cin[name] = nc.declare_dram_parameter(name, list(consts[name].shape), BF16, isOutput=False)
    for name in _F32_CONSTS:
        cin[name] = nc.declare_dram_parameter(name, list(consts[name].shape), F32, isOutput=False)

    ov = nc.declare_dram_parameter("ov", [2, BC], F32, isOutput=True)
    xs_dram = nc.dram_tensor("xs_stash", [4, NT], BF16)

    with tile.TileContext(nc) as tc:
        with (
            tc.tile_pool(name="singles", bufs=1) as sing,
            tc.tile_pool(name="io_p", bufs=4) as io_p,
            tc.tile_pool(name="big", bufs=2) as big,
        ):
            ct = {}
            for name in _BF16_CONSTS + _F32_CONSTS:
                ct[name] = sing.tile(
                    list(consts[name].shape),
                    BF16 if name in _BF16_CONSTS else F32,
                    name=f"ct_{name}",
                    tag=f"ct_{name}",
                )
                nc.sync.dma_start(out=ct[name], in_=cin[name][:, :])

            X2 = sing.tile([128, 32 * BC], BF16)
            eps128 = sing.tile([128, 1], F32)
            nc.vector.memset(eps128, EPS)
            ones_row = sing.tile([1, BC], BF16)
            nc.vector.memset(ones_row, 1.0)

            # ================= Phase T: trunk =================
            scope_T, _ = nc.enter_named_scope("phaseT", False)
            with (
                tc.tile_pool(name="t_pv", bufs=4, space="PSUM") as ps_pv,
                tc.tile_pool(name="t_ssr", bufs=2, space="PSUM") as ps_ssr,
                tc.tile_pool(name="t_lnt", bufs=2, space="PSUM") as ps_lnt,
            ):

                def t_sq(pv, lidx, j):
                    vsq = big.tile([128, BC], BF16, name=f"vsq{lidx}_{j}", tag="vsq")
                    if (lidx == 0 and consts["b1_nz"]) or (lidx == 1 and consts["b2_nz"]):
                        nc.scalar.activation(vsq, pv, AF.Square,
                                             bias=ct["bstk"][:, lidx : lidx + 1])
                    else:
                        nc.scalar.activation(vsq, pv, AF.Square)
                    return vsq

                def t_ssr(vsq, lidx, j):
                    ssr = ps_ssr.tile([128, BC], F32, name=f"ssr{lidx}_{j}", tag="ssr")
                    nc.tensor.matmul(ssr, ct["REP"], vsq, start=True, stop=True)
                    return ssr

                def t_ln(ssr, lidx, j):
                    lnt = ps_lnt.tile([128, BC], F32, name=f"lnt{lidx}_{j}", tag="lnt")
                    nc.scalar.activation(lnt, ssr, AF.Ln, bias=eps128, scale=1.0 / 64.0)
                    return lnt

                def t_exp(lnt, lidx, j):
                    rstd = big.tile([128, BC], F32, name=f"rstd{lidx}_{j}", tag="rstd")
                    nc.scalar.activation(rstd, lnt, AF.Exp, scale=-0.5)
                    return rstd

                def relu_norm(out_ap, pv, rstd, lidx, j):
                    triv = consts["g1_triv"] if lidx == 0 else consts["g2_triv"]
                    gbe = ct["g1be1"] if lidx == 0 else ct["g2be2"]
                    bias_nz = consts["b1_nz"] if lidx == 0 else consts["b2_nz"]
                    if triv and not bias_nz:
                        nc.vector.scalar_tensor_tensor(
                            out=out_ap, in0=pv, scalar=0.0, in1=rstd,
                            op0=ALU.max, op1=ALU.mult,
                        )
                    else:
                        t1 = big.tile([128, BC], F32, name=f"t1_{lidx}_{j}", tag="t1")
                        nc.vector.scalar_tensor_tensor(
                            out=t1, in0=pv,
                            scalar=ct["bstk"][:, lidx : lidx + 1] if bias_nz else 0.0,
                            in1=rstd,
                            op0=ALU.add if bias_nz else ALU.max, op1=ALU.mult,
                        )
                        nc.scalar.activation(
                            out_ap, t1, AF.Relu,
                            bias=gbe[:, 1:2], scale=gbe[:, 0:1],
                        )

                # two iterations interleaved per loop body: engine FIFOs
                # alternate between independent iterations, hiding each
                # stage's upstream-matmul wait behind the sibling's work
                for jp in range(0, 32, 2):
                    js = (jp, jp + 1)
                    st = {j: {} for j in js}
                    for j in js:
                        ta, tb = j, 63 - j
                        oa = io_p.tile([128, BC], BF16, name=f"oa{j}", tag="obs")
                        nc.sync.dma_start(out=oa, in_=obsT[:, ta * BC : (ta + 1) * BC])
                        ob = io_p.tile([128, BC], BF16, name=f"ob{j}", tag="obs")
                        nc.sync.dma_start(out=ob, in_=obsT[:, tb * BC : (tb + 1) * BC])
                        at = io_p.tile([64, BC], BF16, name=f"at{j}", tag="act")
                        nc.sync.dma_start(out=at[0:32, :], in_=actT[:, ta * BC : (ta + 1) * BC])
                        nc.sync.dma_start(out=at[32:64, :], in_=actT[:, tb * BC : (tb + 1) * BC])
                        st[j]["oa"], st[j]["ob"], st[j]["at"] = oa, ob, at
                    for j in js:
                        pv = ps_pv.tile([128, BC], F32, name=f"pv{j}", tag="pv")
                        nc.tensor.matmul(pv[0:64, :], ct["W1c"], st[j]["oa"], start=True,
                                         stop=True, tile_position=(0, 0))
                        nc.tensor.matmul(pv[64:128, :], ct["W1c"], st[j]["ob"], start=True,
                                         stop=True, tile_position=(0, 64))
                        st[j]["pv"] = pv
                    for j in js:
                        st[j]["vsq"] = t_sq(st[j]["pv"], 0, j)
                    for j in js:
                        st[j]["ssr"] = t_ssr(st[j]["vsq"], 0, j)
                    for j in js:
                        st[j]["lnt"] = t_ln(st[j]["ssr"], 0, j)
                    for j in js:
                        st[j]["rstd"] = t_exp(st[j]["lnt"], 0, j)
                    for j in js:
                        xn = big.tile([128, BC], BF16, name=f"xn{j}", tag="xn")
                        relu_norm(xn, st[j]["pv"], st[j]["rstd"], 0, j)
                        st[j]["xn"] = xn
                    for j in js:
                        pv2 = ps_pv.tile([128, BC], F32, name=f"pv2_{j}", tag="pv")
                        nc.tensor.matmul(pv2, ct["W2X"], st[j]["xn"], start=True, stop=False)
                        nc.tensor.matmul(pv2, ct["W2A"], st[j]["at"], start=False, stop=True)
                        st[j]["pv2"] = pv2
                    for j in js:
                        st[j]["vsq2"] = t_sq(st[j]["pv2"], 1, j)
                    for j in js:
                        st[j]["ssr2"] = t_ssr(st[j]["vsq2"], 1, j)
                    for j in js:
                        st[j]["lnt2"] = t_ln(st[j]["ssr2"], 1, j)
                    for j in js:
                        st[j]["rstd2"] = t_exp(st[j]["lnt2"], 1, j)
                    for j in js:
                        relu_norm(X2[:, j * BC : (j + 1) * BC], st[j]["pv2"],
                                  st[j]["rstd2"], 1, j)

            nc.leave_named_scope("phaseT", scope_T, False)
            tc.strict_bb_all_engine_barrier()

            # ================= Phase L: LSTM =================
            scope_L, _ = nc.enter_named_scope("phaseL", False)
            xh_f = sing.tile([128, BC], BF16)
            xh_b = sing.tile([128, BC], BF16)
            cst = sing.tile([128, BC], F32)
            nc.vector.memset(xh_f[0:64, :], 0.0)
            nc.vector.memset(xh_b[0:64, :], 0.0)
            nc.vector.memset(cst, 0.0)
            S = sing.tile([128, 3 * BC], BF16)  # sigmoid(i, f, o)
            tJ = sing.tile([128, BC], BF16)  # tanh(j)
            u = sing.tile([128, BC], BF16)
            cf = sing.tile([128, BC], F32)
            tcl = sing.tile([128, BC], BF16)

            lstm_ps_cm = tc.tile_pool(name="lstm_ps", bufs=1, space="PSUM")
            lstm_ps = lstm_ps_cm.__enter__()
            PG = lstm_ps.tile([128, 4 * BC], F32, tag="PG")
            warm = lstm_ps.tile([1, 64], F32, tag="warm")
            ph4 = [lstm_ps.tile([4, BC], F32, name=f"ph4_{i}", tag=f"ph4_{i}") for i in range(2)]
            stg = [sing.tile([4, BC], BF16, name=f"stg{i}", tag=f"stg{i}") for i in range(2)]

            def xsrc(t_needed):
                if t_needed < 32:
                    return X2[0:64, t_needed * BC : (t_needed + 1) * BC]
                jj = 63 - t_needed
                return X2[64:128, jj * BC : (jj + 1) * BC]

            def head_dots(t):
                ph = ph4[t % 2]
                nc.tensor.matmul(ph, ct["W4F"], xh_f[0:64, :], start=True, stop=False)
                nc.tensor.matmul(ph, ct["W4B"], xh_b[0:64, :], start=False, stop=True)
                sg = stg[t % 2]
                nc.vector.tensor_copy(sg, ph)
                nc.sync.dma_start(out=xs_dram[:, t * BC : (t + 1) * BC], in_=sg)

            # prefetch x for t=0
            nc.sync.dma_start(out=xh_f[64:128, :], in_=xsrc(0))
            nc.sync.dma_start(out=xh_b[64:128, :], in_=xsrc(63))

            for t in range(T):
                for s in range(4):
                    sl = slice(s * BC, (s + 1) * BC)
                    bias_nz = consts["bias_nz"][s]
                    nc.tensor.matmul(PG[0:64, sl], ct[f"Wgf{s}"], xh_f, start=True,
                                     stop=not bias_nz, tile_position=(0, 0))
                    nc.tensor.matmul(PG[64:128, sl], ct[f"Wgb{s}"], xh_b, start=True,
                                     stop=not bias_nz, tile_position=(0, 64))
                    if bias_nz:
                        nc.tensor.matmul(PG[:, sl], ct[f"bias{s}"], ones_row,
                                         start=False, stop=True)
                # head dots of previous step run off the critical path (PE is
                # idle during sigma/cell ops); they read h(t-1) before the
                # h-mul below overwrites it (Tile WAR dep).
                if t > 0:
                    head_dots(t - 1)
                # x prefetch for t+1 via DMA, after this step's gate matmuls
                if t + 1 < T:
                    nc.sync.dma_start(out=xh_f[64:128, :], in_=xsrc(t + 1))
                    nc.sync.dma_start(out=xh_b[64:128, :], in_=xsrc(62 - t))
                # slots: 0=f (forget bias via per-partition ACT bias), 1=i, 2=o, 3=j
                nc.scalar.activation(S[:, 0:BC], PG[:, 0:BC], AF.Sigmoid,
                                     bias=ct["fbias"][:, 0:1])
                nc.scalar.activation(S[:, BC : 3 * BC], PG[:, BC : 3 * BC], AF.Sigmoid)
                nc.scalar.activation(tJ, PG[:, 3 * BC : 4 * BC], AF.Tanh)
                nc.gpsimd.tensor_tensor(out=cf, in0=cst, in1=S[:, 0:BC], op=ALU.mult)
                # keep-warm: PE dummy matmuls mid-step so HAM doesn't rethrottle
                nc.tensor.matmul(warm, S[0:1, 0:1], ones_row[0:1, 0:64],
                                 start=True, stop=True)
                nc.vector.tensor_tensor(out=u, in0=S[:, BC : 2 * BC], in1=tJ, op=ALU.mult)
                nc.vector.tensor_tensor(out=cst, in0=cf, in1=u, op=ALU.add)
                nc.tensor.matmul(warm, u[0:1, 0:1], ones_row[0:1, 0:64],
                                 start=True, stop=True)
                nc.scalar.activation(tcl, cst, AF.Tanh)
                nc.tensor.matmul(warm, tcl[0:1, 0:1], ones_row[0:1, 0:64],
                                 start=True, stop=True)
                nc.vector.tensor_tensor(out=xh_f[0:64, :], in0=tcl[0:64, :],
                                        in1=S[0:64, 2 * BC : 3 * BC], op=ALU.mult)
                nc.vector.tensor_tensor(out=xh_b[0:64, :], in0=tcl[64:128, :],
                                        in1=S[64:128, 2 * BC : 3 * BC], op=ALU.mult)
            head_dots(T - 1)

            lstm_ps_cm.__exit__(None, None, None)
            nc.leave_named_scope("phaseL", scope_L, False)
            tc.strict_bb_all_engine_barrier()

            # ================= Phase H: head =================
            scope_H, _ = nc.enter_named_scope("phaseH", False)
            head_ps_cm = tc.tile_pool(name="head_ps", bufs=1, space="PSUM")
            ps_h = head_ps_cm.__enter__()
            eps64 = sing.tile([64, 1], F32)
            nc.vector.memset(eps64, EPS)
            ovs = [sing.tile([1, BC], F32, name=f"ovs{d}", tag=f"ovs{d}") for d in range(2)]
            for d, (w3, b3, gpb) in enumerate(
                (("W3f", "b3f", "gpbepf"), ("W3b", "b3b", "gpbepb"))
            ):
                xsT = big.tile([64, BC], BF16, name=f"xsT{d}", tag="hxs")
                psT = big.tile([64, BC], BF16, name=f"psT{d}", tag="hps")
                nc.sync.dma_start(
                    out=xsT,
                    in_=xs_dram[2 * d : 2 * d + 1, :].rearrange("o (t b) -> (o t) b", b=BC),
                )
                nc.sync.dma_start(
                    out=psT,
                    in_=xs_dram[2 * d + 1 : 2 * d + 2, :].rearrange("o (t b) -> (o t) b", b=BC),
                )
                pc = ps_h.tile([64, BC], F32, name=f"pc{d}", tag="hpc")
                nc.tensor.matmul(pc, ct["CT"], psT, start=True, stop=True)
                hsq = big.tile([64, BC], BF16, name=f"hsq{d}", tag="vsq")
                nc.scalar.activation(hsq, pc, AF.Square)
                ssrh = ps_h.tile([64, BC], F32, name=f"ssrh{d}", tag="hssr")
                nc.tensor.matmul(ssrh, ct["REP64"], hsq, start=True, stop=True)
                lnh = ps_h.tile([64, BC], F32, name=f"lnh{d}", tag="hlnt")
                nc.scalar.activation(lnh, ssrh, AF.Ln, bias=eps64, scale=1.0 / 64.0)
                preph = big.tile([64, BC], F32, name=f"preph{d}", tag="rstd")
                nc.scalar.activation(preph, lnh, AF.Exp, scale=-0.5)
                pn = big.tile([64, BC], BF16, name=f"pn{d}", tag="xn")
                if consts["gp_triv"]:
                    nc.vector.scalar_tensor_tensor(
                        out=pn, in0=pc, scalar=0.0, in1=preph, op0=ALU.max, op1=ALU.mult
                    )
                else:
                    t1 = big.tile([64, BC], F32, name=f"ht1{d}", tag="t1")
                    nc.vector.scalar_tensor_tensor(
                        out=t1, in0=pc, scalar=1.0, in1=preph, op0=ALU.mult, op1=ALU.mult
                    )
                    nc.scalar.activation(pn, t1, AF.Relu, bias=ct[gpb][:, 1:2],
                                         scale=ct[gpb][:, 0:1])
                pl = ps_h.tile([64, BC], F32, name=f"pl{d}", tag="hpl")
                nc.tensor.matmul(pl, ct[w3], pn, start=True, stop=True)
                he = big.tile([64, BC], BF16, name=f"he{d}", tag="he")
                nc.scalar.activation(he, pl, AF.Exp, bias=ct[b3][:, 0:1])
                hse = ps_h.tile([1, BC], F32, name=f"hse{d}", tag="hse")
                nc.tensor.matmul(hse, ct["ones64"], he, start=True, stop=True)
                lnse = ps_h.tile([1, BC], F32, name=f"lnse{d}", tag="hlns")
                nc.scalar.activation(lnse, hse, AF.Ln)
                hrs = big.tile([1, BC], F32, name=f"hrs{d}", tag="hrs")
                nc.scalar.activation(hrs, lnse, AF.Exp, scale=-1.0)
                hex_ = big.tile([64, BC], BF16, name=f"hex{d}", tag="hex")
                nc.vector.tensor_tensor(out=hex_, in0=he, in1=xsT, op=ALU.mult)
                hnum = ps_h.tile([1, BC], F32, name=f"hnum{d}", tag="hnum")
                nc.tensor.matmul(hnum, ct["ones64"], hex_, start=True, stop=True)
                hov = big.tile([1, BC], F32, name=f"hov{d}", tag="hov")
                nc.vector.tensor_tensor(out=hov, in0=hnum, in1=hrs, op=ALU.mult)
                nc.vector.tensor_scalar(out=ovs[d], in0=hov, scalar1=float(consts["bx"]),
                                        scalar2=None, op0=ALU.add)
                nc.sync.dma_start(out=ov[d : d + 1, :], in_=ovs[d])
            head_ps_cm.__exit__(None, None, None)
            nc.leave_named_scope("phaseH", scope_H, False)

    return nc


_CACHE = {}


def kernel(**inputs):
    obs = np.ascontiguousarray(inputs["obs"], dtype=np.float32)
    action = np.ascontiguousarray(inputs["action"], dtype=np.float32)
    consts = _prep_consts(inputs)

    key = "nc"
    if key not in _CACHE:
        _CACHE[key] = _install_waitfix(_build(consts))
    nc = _CACHE[key]

    const_feed = {}
    for name in _BF16_CONSTS + _F32_CONSTS:
        const_feed[name] = consts[name]

    in_maps = []
    for c in range(NCORES):
        sl = slice(c * BC * T, (c + 1) * BC * T)
        obsT = np.ascontiguousarray(
            obs[sl].reshape(BC, T, OBS).transpose(2, 1, 0).reshape(OBS, NT)
        ).astype(BF)
        actTm = np.ascontiguousarray(
            action[sl].reshape(BC, T, ACTD).transpose(2, 1, 0).reshape(ACTD, NT)
        ).astype(BF)
        m = {"obsT": obsT, "actT": actTm}
        m.update(const_feed)
        in_maps.append(m)

    kw = {}
    if os.environ.get("BASS_TRACE"):
        kw["trace"] = True
        if os.environ.get("BASS_TRACE_DIR"):
            kw["tmpdir"] = os.environ["BASS_TRACE_DIR"]
    full = run_bass_kernel_spmd(nc, in_maps, list(range(NCORES)), **kw)
    globals()["LAST_RESULTS"] = full
    res = full.results

    out = np.empty(2 * B, dtype=np.float32)
    for c in range(NCORES):
        ovc = res[c]["ov"]
        out[c * BC : (c + 1) * BC] = ovc[0]
        out[B + c * BC : B + (c + 1) * BC] = ovc[1]
    return out


# revision 27
# speedup vs baseline: 2.1736x; 1.0859x over previous
"""Trainium2 Bass kernel for nn_Critic (bidirectional-LSTM critic network).

v2: all matmuls bf16 (1 cyc/row vs fp32's 4), LayerNorm via replicated
sum-of-squares matmul + Ln/Exp rsqrt (kills the 3.3us InstReciprocal),
relu+normalize fused into one DVE scalar_tensor_tensor, LSTM gates as a
single fused Sigmoid over a 4-bank PSUM tile (tanh(j) = 2*sigmoid(2j)-1
folded into weights), forget bias via K=1 accumulating matmul, cell-state
mul offloaded to GpSimd, head stash in bf16 with softmax reciprocal as
exp(-ln(x)).

Data-parallel over B: 8 NeuronCores x 512 sequences. Feature-major layout;
trunk processes timestep pairs (j, 63-j) packed on partitions [ta|tb].
"""

import os
import sys

for _p in ("/opt/trn_rl_repo",):
    if _p not in sys.path:
        sys.path.insert(0, _p)

import json as _json
from types import MethodType as _MethodType

import ml_dtypes
import numpy as np

import concourse.bass as bass
import concourse.tile as tile
from concourse import mybir
from concourse.bass_utils import run_bass_kernel_spmd

F32 = mybir.dt.float32
BF16 = mybir.dt.bfloat16
AF = mybir.ActivationFunctionType
ALU = mybir.AluOpType
BF = ml_dtypes.bfloat16

B, T, H, OBS, ACTD = 4096, 64, 64, 128, 32
NCORES = 8
BC = B // NCORES  # 512 sequences per core
NT = T * BC
EPS = 1e-12

# ---------------------------------------------------------------- waitfix --
_MAX_WAITS = 1


def _patch_bir(bir):
    n = [0]

    def fresh():
        n[0] += 1
        return f"I-waitfix-{n[0]}"

    for fn in bir.get("functions", []):
        for bb in fn.get("blocks", []):
            out = []
            for inst in bb.get("instructions", []):
                si = inst.get("sync_info") or {}
                waits = si.get("on_wait") or []
                if len(waits) > _MAX_WAITS:
                    extra = waits[: len(waits) - _MAX_WAITS]
                    keep = waits[len(waits) - _MAX_WAITS :]
                    for i in range(0, len(extra), _MAX_WAITS):
                        out.append(
                            {
                                "name": fresh(),
                                "opcode": "NoOp",
                                "engine": inst["engine"],
                                "ins": [],
                                "outs": [],
                                "sync_info": {
                                    "on_wait": extra[i : i + _MAX_WAITS],
                                    "on_update": [],
                                },
                            }
                        )
                    si = dict(si)
                    si["on_wait"] = keep
                    inst = dict(inst)
                    inst["sync_info"] = si
                out.append(inst)
            bb["instructions"] = out
    return bir


def _install_waitfix(nc):
    orig = nc.to_json_bytes

    def patched(self):
        return _json.dumps(_patch_bir(_json.loads(orig()))).encode()

    nc.to_json_bytes = _MethodType(patched, nc)
    return nc


# ----------------------------------------------------------- host weights --


def _prep_consts(p):
    f32 = lambda x: np.ascontiguousarray(x, dtype=np.float32)
    bf = lambda x: np.ascontiguousarray(np.asarray(x, np.float64), dtype=np.float32).astype(BF)
    C = np.eye(64, dtype=np.float64) - 1.0 / 64.0

    c = {}
    c["W1c"] = bf(np.asarray(p["W1"], np.float64) @ C)  # [128, 64]
    W2c = np.asarray(p["W2"], np.float64) @ C  # [96, 64]
    blk = np.zeros((128, 128), np.float64)
    blk[0:64, 0:64] = W2c[0:64]
    blk[64:128, 64:128] = W2c[0:64]
    c["W2X"] = bf(blk)
    blka = np.zeros((64, 128), np.float64)
    blka[0:32, 0:64] = W2c[64:96]
    blka[32:64, 64:128] = W2c[64:96]
    c["W2A"] = bf(blka)
    rep = np.zeros((128, 128), np.float64)
    rep[0:64, 0:64] = 1.0
    rep[64:128, 64:128] = 1.0
    c["REP"] = bf(rep)

    # trunk fast-path flags and general-path params
    c["b1_nz"] = bool(np.any(np.asarray(p["b1"])))
    c["b2_nz"] = bool(np.any(np.asarray(p["b2"])))
    b1c = np.asarray(p["b1"], np.float64) @ C
    b2c = np.asarray(p["b2"], np.float64) @ C
    c["bstk"] = f32(np.stack([np.tile(b1c, 2), np.tile(b2c, 2)], 1))  # [128,2]
    c["g1_triv"] = bool(
        np.all(np.asarray(p["g1"]) == 1) and np.all(np.asarray(p["be1"]) == 0)
    )
    c["g2_triv"] = bool(
        np.all(np.asarray(p["g2"]) == 1) and np.all(np.asarray(p["be2"]) == 0)
    )
    c["g1be1"] = f32(np.stack([np.tile(p["g1"], 2), np.tile(p["be1"], 2)], 1))
    c["g2be2"] = f32(np.stack([np.tile(p["g2"], 2), np.tile(p["be2"], 2)], 1))

    # LSTM gate weights: lhsT rows = [h (64); x (64)].
    # PG bank order (i, f, o, j): slots 0..2 share one fused Sigmoid,
    # slot 3 gets its own Tanh.
    Wf = np.asarray(p["Wf"], np.float64)
    Wb = np.asarray(p["Wb"], np.float64)
    bfv = np.asarray(p["bf"], np.float64).copy()
    bbv = np.asarray(p["bb"], np.float64).copy()
    bfv[128:192] += 1.0  # forget_bias on f gate
    bbv[128:192] += 1.0
    gb = []
    for slot, g in enumerate((2, 0, 3, 1)):  # slots f,i,o,j <- ref gates i,j,f,o
        sl = slice(g * 64, (g + 1) * 64)
        c[f"Whf{slot}"] = bf(Wf[64:128, sl])
        c[f"Whb{slot}"] = bf(Wb[64:128, sl])
        c[f"Wxf{slot}"] = bf(Wf[0:64, sl])
        c[f"Wxb{slot}"] = bf(Wb[0:64, sl])
        gb.append(np.concatenate([bfv[sl], bbv[sl]]))
    c["bias_nz"] = [bool(np.any(gb[s])) for s in range(4)]
    for s in range(4):
        c[f"bias{s}"] = bf(gb[s].reshape(1, 128))

    wx = np.asarray(p["wx"], np.float64)
    wp = np.asarray(p["wp"], np.float64)
    w4f = np.zeros((64, 4), np.float64)
    w4f[:, 0] = wx
    w4f[:, 1] = wp
    w4b = np.zeros((64, 4), np.float64)
    w4b[:, 2] = wx
    w4b[:, 3] = wp
    c["W4F"] = bf(w4f)
    c["W4B"] = bf(w4b)

    c["CT"] = bf(np.eye(64) - 1.0 / 64.0)
    c["REP64"] = bf(np.ones((64, 64)))
    c["ones64"] = bf(np.ones((64, 1)))
    c["W3f"] = bf(p["W3"])
    c["W3b"] = bf(np.asarray(p["W3"])[::-1, ::-1])
    c["b3f"] = f32(np.asarray(p["b3"]).reshape(64, 1))
    c["b3b"] = f32(np.asarray(p["b3"])[::-1].reshape(64, 1))
    c["gp_triv"] = bool(
        np.all(np.asarray(p["gp"]) == 1) and np.all(np.asarray(p["bep"]) == 0)
    )
    c["gpbepf"] = f32(np.stack([p["gp"], p["bep"]], 1))
    c["gpbepb"] = f32(np.stack([np.asarray(p["gp"])[::-1], np.asarray(p["bep"])[::-1]], 1))
    c["bx"] = float(np.asarray(p["bx"]))
    return c


# ------------------------------------------------------------ bass program --

_BF16_CONSTS = (
    ["W1c", "W2X", "W2A", "REP", "W4F", "W4B", "CT", "REP64", "ones64", "W3f", "W3b"]
    + [f"W{xy}{fb}{g}" for xy in "xh" for fb in "fb" for g in range(4)]
    + [f"bias{g}" for g in range(4)]
)
_F32_CONSTS = ["bstk", "g1be1", "g2be2", "b3f", "b3b", "gpbepf", "gpbepb"]


def _build(consts):
    nc = bass.Bass()
    obsT = nc.declare_dram_parameter("obsT", [128, NT], BF16, isOutput=False)
    actT = nc.declare_dram_parameter("actT", [32, NT], BF16, isOutput=False)

    cin = {}
    for name in _BF16_CONSTS:
        cin[name] = nc.declare_dram_parameter(name, list(consts[name].shape), BF16, isOutput=False)
    for name in _F32_CONSTS:
        cin[name] = nc.declare_dram_parameter(name, list(consts[name].shape), F32, isOutput=False)

    ov = nc.declare_dram_parameter("ov", [2, BC], F32, isOutput=True)
    xs_dram = nc.dram_tensor("xs_stash", [4, NT], BF16)

    with tile.TileContext(nc) as tc:
        with (
            tc.tile_pool(name="singles", bufs=1) as sing,
            tc.tile_pool(name="io_p", bufs=4) as io_p,
            tc.tile_pool(name="big", bufs=2) as big,
        ):
            ct = {}
            for name in _BF16_CONSTS + _F32_CONSTS:
                ct[name] = sing.tile(
                    list(consts[name].shape),
                    BF16 if name in _BF16_CONSTS else F32,
                    name=f"ct_{name}",
                    tag=f"ct_{name}",
                )
                nc.sync.dma_start(out=ct[name], in_=cin[name][:, :])

            X2 = sing.tile([128, 32 * BC], BF16)
            eps128 = sing.tile([128, 1], F32)
            nc.vector.memset(eps128, EPS)
            ones_row = sing.tile([1, BC], BF16)
            nc.vector.memset(ones_row, 1.0)

            # ================= Phase T: trunk =================
            scope_T, _ = nc.enter_named_scope("phaseT", False)
            with (
                tc.tile_pool(name="t_pv", bufs=4, space="PSUM") as ps_pv,
                tc.tile_pool(name="t_ssr", bufs=2, space="PSUM") as ps_ssr,
                tc.tile_pool(name="t_lnt", bufs=2, space="PSUM") as ps_lnt,
            ):

                def t_sq(pv, lidx, j):
                    vsq = big.tile([128, BC], BF16, name=f"vsq{lidx}_{j}", tag="vsq")
                    if (lidx == 0 and consts["b1_nz"]) or (lidx == 1 and consts["b2_nz"]):
                        nc.scalar.activation(vsq, pv, AF.Square,
                                             bias=ct["bstk"][:, lidx : lidx + 1])
                    else:
                        nc.scalar.activation(vsq, pv, AF.Square)
                    return vsq

                def t_ssr(vsq, lidx, j):
                    ssr = ps_ssr.tile([128, BC], F32, name=f"ssr{lidx}_{j}", tag="ssr")
                    nc.tensor.matmul(ssr, ct["REP"], vsq, start=True, stop=True)
                    return ssr

                def t_ln(ssr, lidx, j):
                    lnt = ps_lnt.tile([128, BC], F32, name=f"lnt{lidx}_{j}", tag="lnt")
                    nc.scalar.activation(lnt, ssr, AF.Ln, bias=eps128, scale=1.0 / 64.0)
                    return lnt

                def t_exp(lnt, lidx, j):
                    rstd = big.tile([128, BC], F32, name=f"rstd{lidx}_{j}", tag="rstd")
                    nc.scalar.activation(rstd, lnt, AF.Exp, scale=-0.5)
                    return rstd

                def relu_norm(out_ap, pv, rstd, lidx, j):
                    triv = consts["g1_triv"] if lidx == 0 else consts["g2_triv"]
                    gbe = ct["g1be1"] if lidx == 0 else ct["g2be2"]
                    bias_nz = consts["b1_nz"] if lidx == 0 else consts["b2_nz"]
                    if triv and not bias_nz:
                        nc.vector.scalar_tensor_tensor(
                            out=out_ap, in0=pv, scalar=0.0, in1=rstd,
                            op0=ALU.max, op1=ALU.mult,
                        )
                    else:
                        t1 = big.tile([128, BC], F32, name=f"t1_{lidx}_{j}", tag="t1")
                        nc.vector.scalar_tensor_tensor(
                            out=t1, in0=pv,
                            scalar=ct["bstk"][:, lidx : lidx + 1] if bias_nz else 0.0,
                            in1=rstd,
                            op0=ALU.add if bias_nz else ALU.max, op1=ALU.mult,
                        )
                        nc.scalar.activation(
                            out_ap, t1, AF.Relu,
                            bias=gbe[:, 1:2], scale=gbe[:, 0:1],
                        )

                # two iterations interleaved per loop body: engine FIFOs
                # alternate between independent iterations, hiding each
                # stage's upstream-matmul wait behind the sibling's work
                for jp in range(0, 32, 2):
                    js = (jp, jp + 1)
                    st = {j: {} for j in js}
                    for j in js:
                        ta, tb = j, 63 - j
                        oa = io_p.tile([128, BC], BF16, name=f"oa{j}", tag="obs")
                        nc.sync.dma_start(out=oa, in_=obsT[:, ta * BC : (ta + 1) * BC])
                        ob = io_p.tile([128, BC], BF16, name=f"ob{j}", tag="obs")
                        nc.sync.dma_start(out=ob, in_=obsT[:, tb * BC : (tb + 1) * BC])
                        at = io_p.tile([64, BC], BF16, name=f"at{j}", tag="act")
                        nc.sync.dma_start(out=at[0:32, :], in_=actT[:, ta * BC : (ta + 1) * BC])
                        nc.sync.dma_start(out=at[32:64, :], in_=actT[:, tb * BC : (tb + 1) * BC])
                        st[j]["oa"], st[j]["ob"], st[j]["at"] = oa, ob, at
                    for j in js:
                        pv = ps_pv.tile([128, BC], F32, name=f"pv{j}", tag="pv")
                        nc.tensor.matmul(pv[0:64, :], ct["W1c"], st[j]["oa"], start=True,
                                         stop=True, tile_position=(0, 0))
                        nc.tensor.matmul(pv[64:128, :], ct["W1c"], st[j]["ob"], start=True,
                                         stop=True, tile_position=(0, 64))
                        st[j]["pv"] = pv
                    for j in js:
                        st[j]["vsq"] = t_sq(st[j]["pv"], 0, j)
                    for j in js:
                        st[j]["ssr"] = t_ssr(st[j]["vsq"], 0, j)
                    for j in js:
                        st[j]["lnt"] = t_ln(st[j]["ssr"], 0, j)
                    for j in js:
                        st[j]["rstd"] = t_exp(st[j]["lnt"], 0, j)
                    for j in js:
                        xn = big.tile([128, BC], BF16, name=f"xn{j}", tag="xn")
                        relu_norm(xn, st[j]["pv"], st[j]["rstd"], 0, j)
                        st[j]["xn"] = xn
                    for j in js:
                        pv2 = ps_pv.tile([128, BC], F32, name=f"pv2_{j}", tag="pv")
                        nc.tensor.matmul(pv2, ct["W2X"], st[j]["xn"], start=True, stop=False)
                        nc.tensor.matmul(pv2, ct["W2A"], st[j]["at"], start=False, stop=True)
                        st[j]["pv2"] = pv2
                    for j in js:
                        st[j]["vsq2"] = t_sq(st[j]["pv2"], 1, j)
                    for j in js:
                        st[j]["ssr2"] = t_ssr(st[j]["vsq2"], 1, j)
                    for j in js:
                        st[j]["lnt2"] = t_ln(st[j]["ssr2"], 1, j)
                    for j in js:
                        st[j]["rstd2"] = t_exp(st[j]["lnt2"], 1, j)
                    for j in js:
                        relu_norm(X2[:, j * BC : (j + 1) * BC], st[j]["pv2"],
                                  st[j]["rstd2"], 1, j)

            nc.leave_named_scope("phaseT", scope_T, False)
            tc.strict_bb_all_engine_barrier()

            # ================= Phase L: LSTM =================
            scope_L, _ = nc.enter_named_scope("phaseL", False)
            hf = sing.tile([64, BC], BF16)
            hb = sing.tile([64, BC], BF16)
            xf = sing.tile([64, BC], BF16)
            xb = sing.tile([64, BC], BF16)
            cst = sing.tile([128, BC], F32)
            nc.vector.memset(hf, 0.0)
            nc.vector.memset(hb, 0.0)
            nc.vector.memset(cst, 0.0)
            S = sing.tile([128, 3 * BC], BF16)  # sigmoid(i, f, o)
            tJ = sing.tile([128, BC], BF16)  # tanh(j)
            u = sing.tile([128, BC], BF16)
            cf = sing.tile([128, BC], F32)
            tcl = sing.tile([128, BC], BF16)

            lstm_ps_cm = tc.tile_pool(name="lstm_ps", bufs=1, space="PSUM")
            lstm_ps = lstm_ps_cm.__enter__()
            PG = lstm_ps.tile([128, 4 * BC], F32, tag="PG")
            warm = lstm_ps.tile([1, 64], F32, tag="warm")
            ph4 = [lstm_ps.tile([4, BC], F32, name=f"ph4_{i}", tag=f"ph4_{i}") for i in range(2)]
            stg = [sing.tile([4, BC], BF16, name=f"stg{i}", tag=f"stg{i}") for i in range(2)]

            def xsrc(t_needed):
                if t_needed < 32:
                    return X2[0:64, t_needed * BC : (t_needed + 1) * BC]
                jj = 63 - t_needed
                return X2[64:128, jj * BC : (jj + 1) * BC]

            def head_dots(t):
                ph = ph4[t % 2]
                nc.tensor.matmul(ph, ct["W4F"], hf, start=True, stop=False)
                nc.tensor.matmul(ph, ct["W4B"], hb, start=False, stop=True)
                sg = stg[t % 2]
                nc.vector.tensor_copy(sg, ph)
                nc.sync.dma_start(out=xs_dram[:, t * BC : (t + 1) * BC], in_=sg)

            def x_mms(t):
                """x-part of gate pre-activations for step t: runs in the
                previous step's sigma/cell window while the PE is idle."""
                for s in range(4):
                    sl = slice(s * BC, (s + 1) * BC)
                    nc.tensor.matmul(PG[0:64, sl], ct[f"Wxf{s}"], xf,
                                     start=True, stop=False, tile_position=(0, 0))
                    nc.tensor.matmul(PG[64:128, sl], ct[f"Wxb{s}"], xb,
                                     start=True, stop=False, tile_position=(0, 64))
                    if consts["bias_nz"][s]:
                        nc.tensor.matmul(PG[:, sl], ct[f"bias{s}"], ones_row,
                                         start=False, stop=False)

            # prefetch + x-part for t=0
            nc.sync.dma_start(out=xf, in_=xsrc(0))
            nc.sync.dma_start(out=xb, in_=xsrc(63))
            x_mms(0)

            for t in range(T):
                # h-part: the only matmuls on the recurrence critical path
                for s in range(4):
                    sl = slice(s * BC, (s + 1) * BC)
                    nc.tensor.matmul(PG[0:64, sl], ct[f"Whf{s}"], hf,
                                     start=False, stop=True, tile_position=(0, 0))
                    nc.tensor.matmul(PG[64:128, sl], ct[f"Whb{s}"], hb,
                                     start=False, stop=True, tile_position=(0, 64))
                if t > 0:
                    head_dots(t - 1)
                # slots: 0=f, 1=i, 2=o, 3=j. Fused sigma over (f,i) feeds the
                # cell update immediately; j's tanh and o's sigma follow.
                nc.scalar.activation(S[:, 0 : 2 * BC], PG[:, 0 : 2 * BC], AF.Sigmoid)
                nc.scalar.activation(tJ, PG[:, 3 * BC : 4 * BC], AF.Tanh)
                nc.scalar.activation(S[:, 2 * BC : 3 * BC], PG[:, 2 * BC : 3 * BC],
                                     AF.Sigmoid)
                nc.gpsimd.tensor_tensor(out=cf, in0=cst, in1=S[:, 0:BC], op=ALU.mult)
                # x prefetch + x-part matmuls for t+1 (PG free once this
                # step's sigma/tanh have read it)
                if t + 1 < T:
                    nc.sync.dma_start(out=xf, in_=xsrc(t + 1))
                    nc.sync.dma_start(out=xb, in_=xsrc(62 - t))
                    x_mms(t + 1)
                nc.vector.tensor_tensor(out=u, in0=S[:, BC : 2 * BC], in1=tJ, op=ALU.mult)
                nc.vector.tensor_tensor(out=cst, in0=cf, in1=u, op=ALU.add)
                nc.scalar.activation(tcl, cst, AF.Tanh)
                nc.vector.tensor_tensor(out=hf, in0=tcl[0:64, :],
                                        in1=S[0:64, 2 * BC : 3 * BC], op=ALU.mult)
                nc.vector.tensor_tensor(out=hb, in0=tcl[64:128, :],
                                        in1=S[64:128, 2 * BC : 3 * BC], op=ALU.mult)
            head_dots(T - 1)

            lstm_ps_cm.__exit__(None, None, None)
            nc.leave_named_scope("phaseL", scope_L, False)
            tc.strict_bb_all_engine_barrier()

            # ================= Phase H: head =================
            scope_H, _ = nc.enter_named_scope("phaseH", False)
            head_ps_cm = tc.tile_pool(name="head_ps", bufs=1, space="PSUM")
            ps_h = head_ps_cm.__enter__()
            eps64 = sing.tile([64, 1], F32)
            nc.vector.memset(eps64, EPS)
            ovs = [sing.tile([1, BC], F32, name=f"ovs{d}", tag=f"ovs{d}") for d in range(2)]
            for d, (w3, b3, gpb) in enumerate(
                (("W3f", "b3f", "gpbepf"), ("W3b", "b3b", "gpbepb"))
            ):
                xsT = big.tile([64, BC], BF16, name=f"xsT{d}", tag="hxs")
                psT = big.tile([64, BC], BF16, name=f"psT{d}", tag="hps")
                nc.sync.dma_start(
                    out=xsT,
                    in_=xs_dram[2 * d : 2 * d + 1, :].rearrange("o (t b) -> (o t) b", b=BC),
                )
                nc.sync.dma_start(
                    out=psT,
                    in_=xs_dram[2 * d + 1 : 2 * d + 2, :].rearrange("o (t b) -> (o t) b", b=BC),
                )
                pc = ps_h.tile([64, BC], F32, name=f"pc{d}", tag="hpc")
                nc.tensor.matmul(pc, ct["CT"], psT, start=True, stop=True)
                hsq = big.tile([64, BC], BF16, name=f"hsq{d}", tag="vsq")
                nc.scalar.activation(hsq, pc, AF.Square)
                ssrh = ps_h.tile([64, BC], F32, name=f"ssrh{d}", tag="hssr")
                nc.tensor.matmul(ssrh, ct["REP64"], hsq, start=True, stop=True)
                lnh = ps_h.tile([64, BC], F32, name=f"lnh{d}", tag="hlnt")
                nc.scalar.activation(lnh, ssrh, AF.Ln, bias=eps64, scale=1.0 / 64.0)
                preph = big.tile([64, BC], F32, name=f"preph{d}", tag="rstd")
                nc.scalar.activation(preph, lnh, AF.Exp, scale=-0.5)
                pn = big.tile([64, BC], BF16, name=f"pn{d}", tag="xn")
                if consts["gp_triv"]:
                    nc.vector.scalar_tensor_tensor(
                        out=pn, in0=pc, scalar=0.0, in1=preph, op0=ALU.max, op1=ALU.mult
                    )
                else:
                    t1 = big.tile([64, BC], F32, name=f"ht1{d}", tag="t1")
                    nc.vector.scalar_tensor_tensor(
                        out=t1, in0=pc, scalar=1.0, in1=preph, op0=ALU.mult, op1=ALU.mult
                    )
                    nc.scalar.activation(pn, t1, AF.Relu, bias=ct[gpb][:, 1:2],
                                         scale=ct[gpb][:, 0:1])
                pl = ps_h.tile([64, BC], F32, name=f"pl{d}", tag="hpl")
                nc.tensor.matmul(pl, ct[w3], pn, start=True, stop=True)
                he = big.tile([64, BC], BF16, name=f"he{d}", tag="he")
                nc.scalar.activation(he, pl, AF.Exp, bias=ct[b3][:, 0:1])
                hse = ps_h.tile([1, BC], F32, name=f"hse{d}", tag="hse")
                nc.tensor.matmul(hse, ct["ones64"], he, start=True, stop=True)
                lnse = ps_h.tile([1, BC], F32, name=f"lnse{d}", tag="hlns")
                nc.scalar.activation(lnse, hse, AF.Ln)
                hrs = big.tile([1, BC], F32, name=f"hrs{d}", tag="hrs")
                nc.scalar.activation(hrs, lnse, AF.Exp, scale=-1.0)
                hex_ = big.tile([64, BC], BF16, name=f"hex{d}", tag="hex")
                nc.vector.tensor_tensor(out=hex_, in0=he, in1=xsT, op=ALU.mult)
                hnum = ps_h.tile([1, BC], F32, name=f"hnum{d}", tag="hnum")
                nc.tensor.matmul(hnum, ct["ones64"], hex_, start=True, stop=True)
                hov = big.tile([1, BC], F32, name=f"hov{d}", tag="hov")
                nc.vector.tensor_tensor(out=hov, in0=hnum, in1=hrs, op=ALU.mult)
                nc.vector.tensor_scalar(out=ovs[d], in0=hov, scalar1=float(consts["bx"]),
                                        scalar2=None, op0=ALU.add)
                nc.sync.dma_start(out=ov[d : d + 1, :], in_=ovs[d])
            head_ps_cm.__exit__(None, None, None)
            nc.leave_named_scope("phaseH", scope_H, False)

    return nc


_CACHE = {}


def kernel(**inputs):
    obs = np.ascontiguousarray(inputs["obs"], dtype=np.float32)
    action = np.ascontiguousarray(inputs["action"], dtype=np.float32)
    consts = _prep_consts(inputs)

    key = "nc"
    if key not in _CACHE:
        _CACHE[key] = _install_waitfix(_build(consts))
    nc = _CACHE[key]

    const_feed = {}
    for name in _BF16_CONSTS + _F32_CONSTS:
        const_feed[name] = consts[name]

    in_maps = []
    for c in range(NCORES):
        sl = slice(c * BC * T, (c + 1) * BC * T)
        obsT = np.ascontiguousarray(
            obs[sl].reshape(BC, T, OBS).transpose(2, 1, 0).reshape(OBS, NT)
        ).astype(BF)
        actTm = np.ascontiguousarray(
            action[sl].reshape(BC, T, ACTD).transpose(2, 1, 0).reshape(ACTD, NT)
        ).astype(BF)
        m = {"obsT": obsT, "actT": actTm}
        m.update(const_feed)
        in_maps.append(m)

    kw = {}
    if os.environ.get("BASS_TRACE"):
        kw["trace"] = True
        if os.environ.get("BASS_TRACE_DIR"):
            kw["tmpdir"] = os.environ["BASS_TRACE_DIR"]
    full = run_bass_kernel_spmd(nc, in_maps, list(range(NCORES)), **kw)
    globals()["LAST_RESULTS"] = full
    res = full.results

    out = np.empty(2 * B, dtype=np.float32)
    for c in range(NCORES):
        ovc = res[c]["ov"]
        out[c * BC : (c + 1) * BC] = ovc[0]
        out[B + c * BC : B + (c + 1) * BC] = ovc[1]
    return out
